# revision 15
# baseline (speedup 1.0000x reference)
"""Trainium2 Bass kernel for the PaiNN-style GNN message-passing layer.

Strategy (8 NeuronCores, SPMD, no collectives):
- Node rows are block-sharded: core c owns rows [c*NLOC, (c+1)*NLOC).
- Each edge is assigned to the core owning its destination (row) node, so
  all scatter-adds are core-local.
- Per-core compacted endpoint tables (unique cols < 32768) make gather
  indices fit the int16 DMA-gather index format.
- Edge MLP runs in bf16, features-on-partitions; the final MLP layer is
  computed "flipped" (activations as the stationary operand) so messages
  come out edge-major, ready for aggregation without transposes.
- Aggregation is an exact f32 one-hot matmul into a block-persistent PSUM
  tile (one 128-row node block at a time), fused with the output epilogue.
"""

import os
import numpy as np
import ml_dtypes

from concourse import bacc, bass, tile, mybir
from concourse.bass_utils import run_bass_kernel_spmd

BF16 = mybir.dt.bfloat16
F32 = mybir.dt.float32
I16 = mybir.dt.int16
I32 = mybir.dt.int32

HID = 128
NRBF = 32
EDGE = 16
CUTOFF = 5.0
EPS = 1e-8


class Cfg:
    def __init__(self, n_nodes, n_cores, u_max, sub=512):
        self.N = n_nodes
        self.C = n_cores
        self.NLOC = n_nodes // n_cores
        assert self.NLOC * n_cores == n_nodes
        self.NBLK = (self.NLOC + 127) // 128
        self.NLOC_PAD = self.NBLK * 128
        self.U = u_max  # compacted col-table rows (multiple of 128)
        self.SUB = sub  # edge sub-chunk (<=512, PSUM bank limit)


FULL = Cfg(50000, 8, 32768)


def _silu(nc, wrk, use_silu, out, xp, sub, bias, SUB):
    if use_silu:
        nc.scalar.activation(out[:, :sub], xp[:, :sub],
                             mybir.ActivationFunctionType.Silu, bias=bias[:])
    else:  # CoreSim lacks Silu: z*sigmoid(z)
        z = wrk.tile([128, SUB], BF16, tag="slz")
        nc.scalar.activation(z[:, :sub], xp[:, :sub],
                             mybir.ActivationFunctionType.Identity, bias=bias[:])
        sg = wrk.tile([128, SUB], BF16, tag="slg")
        nc.scalar.activation(sg[:, :sub], z[:, :sub],
                             mybir.ActivationFunctionType.Sigmoid)
        nc.vector.tensor_tensor(out[:, :sub], z[:, :sub], sg[:, :sub],
                                mybir.AluOpType.mult)


def _wrap_idx(idx):
    """int16 gather-index layout: [i%16, i//16], replicated 8x to 128 rows."""
    n = len(idx)
    assert n % 16 == 0
    w = idx.astype(np.int16).reshape(n // 16, 16).T
    return np.tile(w, (8, 1)).copy()


def build_graph(cfg, n_slots, use_silu=True, b_mode="full", lean=False):
    """Build the SPMD Bass graph. n_slots: per-block edge-slot counts
    (multiples of 128, identical across cores)."""
    nc = bacc.Bacc(None, target_bir_lowering=False, debug=False)
    NLOC_PAD, U, NBLK, SUB = cfg.NLOC_PAD, cfg.U, cfg.NBLK, cfg.SUB
    ES = int(sum(n_slots))  # total edge slots
    NSUB = ES // 128

    di = lambda name, shape, dt: nc.dram_tensor(name, shape, dt, kind="ExternalInput")
    # node-side uploads
    h_loc = di("h_loc", [NLOC_PAD, 128], BF16)
    h_u = di("h_u", [U, 128], BF16)
    vec_loc_T = di("vec_loc_T", [3, 128, NLOC_PAD], BF16)
    vec_T_u = di("vec_T_u", [3, 128, U], BF16)
    # edge-side uploads
    ucol_w = di("ucol_w", [128, ES // 16], I16)
    urow_w = di("urow_w", [128, ES // 16], I16)
    row_rel = di("row_rel", [128, NSUB], F32)
    xij_t = di("xij_t", [128, NSUB, 3], BF16)
    dense_bf = di("dense_bf", [48, ES], BF16)  # rows 0:32 d_ij, 32:48 edge_attr
    # weights
    wvp = di("wvp", [128, 384], BF16)
    w1p = di("w1p", [4, 128, 128], BF16)
    w2 = di("w2", [128, 128], BF16)
    w3 = di("w3", [128, 384], BF16)
    wop = di("wop", [128, 384], BF16)
    b1 = di("b1", [128, 1], F32)
    b2 = di("b2", [128, 1], F32)
    b3r = di("b3r", [1, 384], F32)
    bop = di("bop", [128, 3], F32)

    dh_o = nc.dram_tensor("dh", [cfg.NLOC, 128], F32, kind="ExternalOutput")
    dvec_o = nc.dram_tensor("dvec", [cfg.NLOC, 384], F32, kind="ExternalOutput")

    # internal DRAM gather tables
    tab_A = nc.dram_tensor("tab_A", [U, 512], BF16)  # [h | vec2_v0 | vec2_v1 | vec2_v2]
    tab_B = nc.dram_tensor("tab_B", [U, 384], BF16)  # vec3 (v,f)
    tab_R = nc.dram_tensor("tab_R", [NLOC_PAD, 512], BF16)  # [h | vec1_v0..v2]

    with tile.TileContext(nc) as tc:
        with (
            tc.tile_pool(name="resident", bufs=1) as res,
            tc.tile_pool(name="stage", bufs=2 if lean else 3) as stg,
            tc.tile_pool(name="blk", bufs=1 if lean else 2) as blk,
            tc.tile_pool(name="work", bufs=2 if lean else 3) as wrk,
            tc.tile_pool(name="psA", bufs=2, space=bass.MemorySpace.PSUM) as psA,
            tc.tile_pool(name="psW", bufs=2, space=bass.MemorySpace.PSUM) as psW,
            tc.tile_pool(name="psM", bufs=2, space=bass.MemorySpace.PSUM) as psM,
            tc.tile_pool(name="psE", bufs=2, space=bass.MemorySpace.PSUM) as psE,
        ):
            # ---- one-time setup ----
            iota_i = res.tile([128, 128], I32)
            nc.gpsimd.iota(iota_i[:], [[1, 128]], channel_multiplier=0)
            iota_colf = res.tile([128, 128], F32)
            nc.vector.tensor_copy(iota_colf[:], iota_i[:])
            iota_p = res.tile([128, 1], I32)
            nc.gpsimd.iota(iota_p[:], [[1, 1]], channel_multiplier=1)
            iota_pf = res.tile([128, 1], F32)
            nc.vector.tensor_copy(iota_pf[:], iota_p[:])
            ident_bf = res.tile([128, 128], BF16)
            nc.vector.tensor_tensor(
                ident_bf[:], iota_colf[:], iota_pf[:].to_broadcast((128, 128)),
                mybir.AluOpType.is_equal)

            ones1 = res.tile([1, 128], F32)
            nc.vector.memset(ones1[:], 1.0)
            b3row = res.tile([1, 384], F32)
            nc.sync.dma_start(b3row[:], b3r[:])
            b3p = psM.tile([128, 512], F32, tag="m")
            nc.tensor.matmul(b3p[:, 0:384], ones1[:], b3row[:])
            b3_bc = res.tile([128, 384], F32)
            nc.scalar.copy(b3_bc[:], b3p[:, 0:384])

            # weights to SBUF
            wvp_s = res.tile([128, 384], BF16)
            nc.sync.dma_start(wvp_s[:], wvp[:])
            w1_s = res.tile([128, 4, 128], BF16)
            nc.sync.dma_start(w1_s[:], w1p[:].rearrange("k p f -> p k f"))
            w2_s = res.tile([128, 128], BF16)
            nc.sync.dma_start(w2_s[:], w2[:])
            w3_s = res.tile([128, 384], BF16)
            nc.sync.dma_start(w3_s[:], w3[:])
            wop_s = res.tile([128, 384], BF16)
            nc.sync.dma_start(wop_s[:], wop[:])
            b1_s = res.tile([128, 1], F32)
            nc.sync.dma_start(b1_s[:], b1[:])
            b2_s = res.tile([128, 1], F32)
            nc.sync.dma_start(b2_s[:], b2[:])
            bop_s = res.tile([128, 3], F32)
            nc.sync.dma_start(bop_s[:], bop[:])

            # resident edge metadata
            ucol_s = res.tile([128, ES // 16], I16)
            nc.sync.dma_start(ucol_s[:], ucol_w[:])
            urow_s = res.tile([128, ES // 16], I16)
            nc.sync.dma_start(urow_s[:], urow_w[:])
            rrel_s = res.tile([128, NSUB], F32)
            nc.sync.dma_start(rrel_s[:], row_rel[:])
            xij_s = res.tile([128, NSUB, 3], BF16)
            nc.sync.dma_start(xij_s[:], xij_t[:])

            # resident node data filled by phase A
            vdot_s = res.tile([128, cfg.NBLK, 128], BF16)
            v3_s = res.tile([128, cfg.NBLK, 384], BF16)

            # ---- phase A: local vecp, vec_dot, vec3, row table ----
            nc.sync.dma_start(
                tab_R[:].rearrange("n (t f) -> n t f", t=4)[:, 0, :], h_loc[:])
            for t in range(NLOC_PAD // 128):
                vlt = stg.tile([128, 3, 128], BF16)
                nc.sync.dma_start(
                    vlt[:],
                    vec_loc_T[:, :, 128 * t:128 * (t + 1)].rearrange(
                        "v p u -> p v u"))
                v1st = stg.tile([128, 3, 128], BF16)
                acc = None
                for v in range(3):
                    p = psM.tile([128, 512], F32, tag="m")
                    nc.tensor.matmul(p[:, 0:384], vlt[:, v, :], wvp_s[:])
                    nc.vector.tensor_copy(v1st[:, v, :], p[:, 0:128])
                    tm = wrk.tile([128, 128], F32)
                    nc.vector.tensor_tensor(tm[:], p[:, 128:256], v1st[:, v, :],
                                            mybir.AluOpType.mult)
                    if v == 0:
                        acc = tm
                    elif v == 1:
                        nc.vector.tensor_tensor(acc[:], acc[:], tm[:],
                                                mybir.AluOpType.add)
                    else:
                        nc.vector.tensor_tensor(vdot_s[:, t, :], acc[:], tm[:],
                                                mybir.AluOpType.add)
                    nc.scalar.copy(v3_s[:, t, 128 * v:128 * (v + 1)],
                                   p[:, 256:384])
                nc.sync.dma_start(
                    tab_R[128 * t:128 * (t + 1), 128:512], v1st[:])

            # ---- phase A2: compacted col tables ----
            nc.sync.dma_start(
                tab_A[:].rearrange("n (t f) -> n t f", t=4)[:, 0, :], h_u[:])
            for g in range(U // 512):  # groups of 4 u-tiles
                vtg = stg.tile([128, 3, 512], BF16)
                nc.sync.dma_start(
                    vtg[:],
                    vec_T_u[:, :, 512 * g:512 * (g + 1)].rearrange(
                        "v p u -> p v u"))
                stA = stg.tile([128, 4, 384], BF16)
                stB = stg.tile([128, 4, 384], BF16)
                for j in range(4):
                    ut = 4 * g + j
                    for v in range(3):
                        p = psM.tile([128, 512], F32, tag="m")
                        nc.tensor.matmul(
                            p[:, 0:256], vtg[:, v, 128 * j:128 * (j + 1)],
                            wvp_s[:, 128:384])
                        nc.vector.tensor_copy(stA[:, j, 128 * v:128 * (v + 1)],
                                              p[:, 0:128])
                        nc.scalar.copy(stB[:, j, 128 * v:128 * (v + 1)],
                                       p[:, 128:256])
                uslc = slice(512 * g, 512 * (g + 1))
                nc.sync.dma_start(
                    tab_A[uslc, 128:512].rearrange("(j p) f -> p j f", p=128),
                    stA[:])
                nc.sync.dma_start(
                    tab_B[uslc, :].rearrange("(j p) f -> p j f", p=128), stB[:])

            # ---- phase B: per node-block edge pipeline + fused epilogue ----
            ones_bf = res.tile([128, 128], BF16)
            nc.vector.memset(ones_bf[:], 1.0)
            zed = res.tile([128, 512], BF16)
            nc.vector.memset(zed[:], 0.0)
            if b_mode == "tables":
                for b in range(NBLK):
                    rows = min(128, cfg.NLOC - 128 * b)
                    t = wrk.tile([128, 64], F32, tag="tt")
                    nc.sync.dma_start(
                        t[:], tab_R[128 * b:128 * (b + 1), 0:128].bitcast(F32))
                    nc.sync.dma_start(dh_o[128 * b:128 * b + rows, 0:64],
                                      t[0:rows, :])
            s_off = 0  # slot offset (multiple of 128)
            for b in range(NBLK if b_mode != "tables" else 0):
                ns = int(n_slots[b])
                if ns == 0:
                    continue

                agg = psA.tile([128, 512], F32)
                nc.tensor.matmul(agg[:], ones_bf[:], zed[:], start=True,
                                 stop=False, skip_group_check=True)
                nq = ns // 128
                for c0 in range(0, ns, SUB):
                    sub = min(SUB, ns - c0)
                    ia = (s_off + c0) // 16
                    ib = (s_off + c0 + sub) // 16
                    gA_t = blk.tile([128, 4 * SUB], BF16, tag="gA")
                    gA = gA_t[:, :4 * sub].rearrange("p (t s) -> p t s", t=4)
                    nc.gpsimd.dma_gather(gA, tab_A[:],
                                         ucol_s[:, ia:ib], sub, sub, 512,
                                         transpose=True)
                    gB_t = blk.tile([128, SUB // 128, 384], BF16, tag="gB")
                    gB = gB_t[:, :sub // 128, :]
                    nc.gpsimd.dma_gather(gB, tab_B[:],
                                         ucol_s[:, ia:ib], sub, sub, 384)
                    gR_t = blk.tile([128, 4 * SUB], BF16, tag="gR")
                    gR = gR_t[:, :4 * sub].rearrange("p (t s) -> p t s", t=4)
                    nc.gpsimd.dma_gather(gR, tab_R[:],
                                         urow_s[:, ia:ib], sub, sub, 512,
                                         transpose=True)
                    # cross = sum_v vec1row_v * vec2col_v  (feature-major)
                    cr = wrk.tile([128, SUB], BF16)
                    tt = wrk.tile([128, SUB], BF16)
                    nc.vector.tensor_tensor(
                        cr[:, :sub], gR[:, 1, :sub], gA[:, 1, :sub],
                        mybir.AluOpType.mult)
                    nc.vector.tensor_tensor(
                        tt[:, :sub], gR[:, 2, :sub], gA[:, 2, :sub],
                        mybir.AluOpType.mult)
                    nc.vector.tensor_tensor(cr[:, :sub], cr[:, :sub], tt[:, :sub],
                                            mybir.AluOpType.add)
                    nc.vector.tensor_tensor(
                        tt[:, :sub], gR[:, 3, :sub], gA[:, 3, :sub],
                        mybir.AluOpType.mult)
                    nc.vector.tensor_tensor(cr[:, :sub], cr[:, :sub], tt[:, :sub],
                                            mybir.AluOpType.add)
                    # k3 = [d_ij(32) | edge_attr(16) | zeros]
                    k3 = wrk.tile([128, SUB], BF16)
                    nc.vector.memset(k3[:, :sub], 0.0)
                    nc.sync.dma_start(k3[0:48, :sub],
                                      dense_bf[:, s_off + c0:s_off + c0 + sub])
                    # L1
                    x1p = psW.tile([128, SUB], F32, tag="w")
                    nc.tensor.matmul(x1p[:, :sub], w1_s[:, 0, :],
                                     gR[:, 0, :sub], start=True, stop=False)
                    nc.tensor.matmul(x1p[:, :sub], w1_s[:, 1, :],
                                     gA[:, 0, :sub], start=False, stop=False)
                    nc.tensor.matmul(x1p[:, :sub], w1_s[:, 2, :], cr[:, :sub],
                                     start=False, stop=False)
                    nc.tensor.matmul(x1p[:, :sub], w1_s[:, 3, :], k3[:, :sub],
                                     start=False, stop=True)
                    x1 = wrk.tile([128, SUB], BF16)
                    _silu(nc, wrk, use_silu, x1, x1p, sub, b1_s, SUB)
                    # L2
                    x2p = psW.tile([128, SUB], F32, tag="w")
                    nc.tensor.matmul(x2p[:, :sub], w2_s[:], x1[:, :sub])
                    x2 = wrk.tile([128, SUB], BF16)
                    _silu(nc, wrk, use_silu, x2, x2p, sub, b2_s, SUB)
                    # L3 flipped + aggregation per 128-edge subtile
                    for ql in range(sub // 128):
                        q = (s_off + c0) // 128 + ql
                        qb = (c0 // 128) + ql
                        mp = psM.tile([128, 512], F32, tag="m")
                        nc.tensor.matmul(mp[:, 0:384],
                                         x2[:, 128 * ql:128 * (ql + 1)], w3_s[:])
                        msg = wrk.tile([128, 384], BF16)
                        nc.vector.tensor_tensor(msg[:], mp[:, 0:384], b3_bc[:],
                                                mybir.AluOpType.add)
                        # vec_msg = vec3col*m_v + x_ij*m_x  (edge-major)
                        t1 = wrk.tile([128, 3, 128], F32)
                        nc.vector.tensor_tensor(
                            t1[:],
                            gB[:, ql, :].rearrange("p (v f) -> p v f", v=3),
                            msg[:, 128:256].rearrange("p (a f) -> p a f", a=1).to_broadcast(
                                (128, 3, 128)),
                            mybir.AluOpType.mult)
                        t2 = wrk.tile([128, 3, 128], F32)
                        nc.vector.tensor_tensor(
                            t2[:],
                            xij_s[:, q, :].rearrange("p (v a) -> p v a", a=1).to_broadcast(
                                (128, 3, 128)),
                            msg[:, 256:384].rearrange("p (a f) -> p a f", a=1).to_broadcast(
                                (128, 3, 128)),
                            mybir.AluOpType.mult)
                        vm = wrk.tile([128, 3, 128], BF16)
                        nc.vector.tensor_tensor(vm[:], t1[:], t2[:],
                                                mybir.AluOpType.add)
                        # one-hot scatter
                        S = wrk.tile([128, 128], BF16)
                        nc.vector.tensor_tensor(
                            S[:], rrel_s[:, q:q + 1].to_broadcast((128, 128)),
                            iota_colf[:], mybir.AluOpType.is_equal)
                        first = False
                        last = qb == nq - 1
                        nc.tensor.matmul(agg[:, 0:128], S[:], msg[:, 0:128],
                                         start=first, stop=last,
                                         skip_group_check=True)
                        nc.tensor.matmul(
                            agg[:, 128:512], S[:],
                            vm[:].rearrange("p v f -> p (v f)"),
                            start=first, stop=last, skip_group_check=True)

                # ---- epilogue for block b ----
                hag = wrk.tile([128, 128], BF16)
                nc.vector.tensor_copy(hag[:], agg[:, 0:128])
                hagT_p = psE.tile([128, 1024], BF16, tag="e")
                nc.tensor.transpose(hagT_p[:, 0:128], hag[:], ident_bf[:])
                hagT = wrk.tile([128, 128], BF16)
                nc.scalar.copy(hagT[:], hagT_p[:, 0:128])
                oT = []
                for j in range(3):
                    op = psE.tile([128, 512], F32, tag="e")
                    nc.tensor.matmul(op[:, 0:128], wop_s[:, 128 * j:128 * (j + 1)],
                                     hagT[:])
                    osb = wrk.tile([128, 128], BF16)
                    nc.scalar.activation(osb[:], op[:, 0:128],
                                         mybir.ActivationFunctionType.Identity,
                                         bias=bop_s[:, j:j + 1])
                    otp = psE.tile([128, 1024], BF16, tag="e")
                    nc.tensor.transpose(otp[:, 0:128], osb[:], ident_bf[:])
                    ot = wrk.tile([128, 128], BF16)
                    nc.scalar.copy(ot[:], otp[:, 0:128])
                    oT.append(ot)
                rows = min(128, cfg.NLOC - 128 * b)
                dht = wrk.tile([128, 128], F32)
                nc.vector.tensor_tensor(dht[:], vdot_s[:, b, :], oT[1][:],
                                        mybir.AluOpType.mult)
                nc.vector.tensor_tensor(dht[:], dht[:], oT[2][:],
                                        mybir.AluOpType.add)
                nc.sync.dma_start(dh_o[128 * b:128 * b + rows, :],
                                  dht[0:rows, :])
                vma = wrk.tile([128, 3, 128], F32)
                nc.vector.tensor_tensor(
                    vma[:], v3_s[:, b, :].rearrange("p (v f) -> p v f", v=3),
                    oT[0][:].rearrange("p (a f) -> p a f", a=1).to_broadcast(
                        (128, 3, 128)),
                    mybir.AluOpType.mult)
                dvt = wrk.tile([128, 3, 128], F32)
                nc.vector.tensor_tensor(
                    dvt[:], vma[:],
                    agg[:, 128:512].rearrange("p (v f) -> p v f", v=3),
                    mybir.AluOpType.add)
                nc.sync.dma_start(dvec_o[128 * b:128 * b + rows, :],
                                  dvt[0:rows, :, :].rearrange("p v f -> p (v f)"))
                s_off += ns

    nc.compile()
    return nc


def host_prep(cfg, h, vec, coord, edge_index, edge_attr,
              Wvp, W1, b1, W2, b2, W3, b3, Wop, bop):
    """Shard + lay out all inputs. Returns (n_slots, in_maps)."""
    C, NLOC, NBLK, U = cfg.C, cfg.NLOC, cfg.NBLK, cfg.U
    bf = ml_dtypes.bfloat16
    row = np.asarray(edge_index[0], np.int64)
    col = np.asarray(edge_index[1], np.int64)
    E = row.shape[0]
    h = np.asarray(h, np.float32)
    vec = np.asarray(vec, np.float32)
    coord = np.asarray(coord, np.float32)
    edge_attr = np.asarray(edge_attr, np.float32)

    x_ij = coord[row] - coord[col]
    d = np.sqrt((x_ij * x_ij).sum(-1) + EPS)
    offs = np.linspace(0.0, CUTOFF, NRBF, dtype=np.float32)
    coeff = np.float32(-0.5 / (offs[1] - offs[0]) ** 2)
    d_ij = np.exp(coeff * (d[:, None] - offs[None, :]) ** 2).astype(np.float32)

    core = row // NLOC
    lrow = (row - core * NLOC).astype(np.int64)
    blk = lrow // 128

    # per-core, per-block edge lists
    per = []
    cnt = np.zeros((C, NBLK), np.int64)
    for c in range(C):
        m = np.nonzero(core == c)[0]
        order = np.argsort(lrow[m], kind="stable")
        e = m[order]
        per.append(e)
        cb = np.bincount(blk[e], minlength=NBLK)
        cnt[c] = cb
    n_slots = (np.ceil(cnt.max(axis=0) / 128).astype(np.int64) * 128)
    n_slots = np.maximum(n_slots, 128)
    ES = int(n_slots.sum())
    NSUB = ES // 128
    starts = np.concatenate([[0], np.cumsum(n_slots)[:-1]])

    # MLP weight repack: k-tiles [h_row, h_col, cross, (d_ij|edge_attr|0)]
    W1 = np.asarray(W1, np.float32)
    w1p = np.zeros((4, 128, 128), np.float32)
    w1p[0] = W1[0:128]
    w1p[1] = W1[128:256]
    w1p[2] = W1[288:416]
    w1p[3, 0:32] = W1[256:288]
    w1p[3, 32:48] = W1[416:432]

    in_maps = []
    for c in range(C):
        e = per[c]
        lr = lrow[e]
        bl = blk[e]
        # slot positions
        pos = np.empty(len(e), np.int64)
        off = np.zeros(NBLK, np.int64)
        # edges are sorted by lrow hence by block; place sequentially per block
        for b in range(NBLK):
            k = np.nonzero(bl == b)[0]
            pos[k] = starts[b] + np.arange(len(k))
        ucols, uinv = np.unique(col[e], return_inverse=True)
        nu = len(ucols)
        assert nu <= U, f"core {c}: {nu} unique cols > {U}"

        ucol_a = np.zeros(ES, np.int64)
        urow_a = np.zeros(ES, np.int64)
        rrel_a = np.full(ES, -1.0, np.float32)
        xij_a = np.zeros((ES, 3), np.float32)
        dense_a = np.zeros((48, ES), np.float32)
        ucol_a[pos] = uinv
        urow_a[pos] = lr
        rrel_a[pos] = (lr - 128 * bl).astype(np.float32)
        xij_a[pos] = x_ij[e]
        dense_a[0:32, pos] = d_ij[e].T
        dense_a[32:48, pos] = edge_attr[e].T

        nlp = cfg.NLOC_PAD
        h_loc = np.zeros((nlp, 128), np.float32)
        h_loc[0:NLOC] = h[c * NLOC:(c + 1) * NLOC]
        vl = np.zeros((3, 128, nlp), np.float32)
        vl[:, :, 0:NLOC] = vec[c * NLOC:(c + 1) * NLOC].transpose(1, 2, 0)
        h_uu = np.zeros((U, 128), np.float32)
        h_uu[0:nu] = h[ucols]
        vtu = np.zeros((3, 128, U), np.float32)
        vtu[:, :, 0:nu] = vec[ucols].transpose(1, 2, 0)

        in_maps.append({
            "h_loc": h_loc.astype(bf),
            "h_u": h_uu.astype(bf),
            "vec_loc_T": vl.astype(bf),
            "vec_T_u": vtu.astype(bf),
            "ucol_w": _wrap_idx(ucol_a),
            "urow_w": _wrap_idx(urow_a),
            "row_rel": rrel_a.reshape(NSUB, 128).T.copy(),
            "xij_t": xij_a.reshape(NSUB, 128, 3).transpose(1, 0, 2).astype(bf),
            "dense_bf": dense_a.astype(bf),
            "wvp": np.asarray(Wvp, np.float32).astype(bf),
            "w1p": w1p.astype(bf),
            "w2": np.asarray(W2, np.float32).astype(bf),
            "w3": np.asarray(W3, np.float32).astype(bf),
            "wop": np.asarray(Wop, np.float32).astype(bf),
            "b1": np.asarray(b1, np.float32).reshape(128, 1),
            "b2": np.asarray(b2, np.float32).reshape(128, 1),
            "b3r": np.asarray(b3, np.float32).reshape(1, 384),
            "bop": np.asarray(bop, np.float32).reshape(3, 128).T.copy(),
        })
    return n_slots, in_maps


_CACHE = {}


def _get_graph(cfg, n_slots):
    key = (cfg.N, cfg.C, tuple(int(x) for x in n_slots))
    if key not in _CACHE:
        _CACHE[key] = build_graph(cfg, n_slots)
    return _CACHE[key]


def kernel(h, vec, coord, edge_index, edge_attr,
           Wvp, W1, b1, W2, b2, W3, b3, Wop, bop):
    cfg = FULL
    n_slots, in_maps = host_prep(cfg, h, vec, coord, edge_index, edge_attr,
                                 Wvp, W1, b1, W2, b2, W3, b3, Wop, bop)
    nc = _get_graph(cfg, n_slots)
    trace = bool(int(os.environ.get("BASS_KERNEL_TRACE", "0")))
    res = run_bass_kernel_spmd(nc, in_maps, list(range(cfg.C)), trace=trace)
    kernel.last_exec_time_ns = res.exec_time_ns
    kernel.last_results = res
    dh = np.concatenate([res.results[c]["dh"] for c in range(cfg.C)], axis=0)
    dvec = np.concatenate([res.results[c]["dvec"] for c in range(cfg.C)],
                          axis=0).reshape(cfg.N, 3, 128)
    return dh, dvec


# revision 17
# speedup vs baseline: 1.2453x; 1.2453x over previous
"""Trainium2 Bass kernel for the PaiNN-style GNN message-passing layer.

Strategy (8 NeuronCores, SPMD, no collectives):
- Node rows are block-sharded: core c owns rows [c*NLOC, (c+1)*NLOC).
- Each edge is assigned to the core owning its destination (row) node, so
  all scatter-adds are core-local.
- Per-core compacted endpoint tables (unique cols < 32768) make gather
  indices fit the int16 DMA-gather index format.
- Edge MLP runs in bf16, features-on-partitions; the final MLP layer is
  computed "flipped" (activations as the stationary operand) so messages
  come out edge-major, ready for aggregation without transposes.
- Aggregation is an exact f32 one-hot matmul into a block-persistent PSUM
  tile (one 128-row node block at a time), fused with the output epilogue.
"""

import os
import numpy as np
import ml_dtypes

from concourse import bacc, bass, tile, mybir
from concourse.bass_utils import run_bass_kernel_spmd

BF16 = mybir.dt.bfloat16
F32 = mybir.dt.float32
I16 = mybir.dt.int16
I32 = mybir.dt.int32

HID = 128
NRBF = 32
EDGE = 16
CUTOFF = 5.0
EPS = 1e-8


class Cfg:
    def __init__(self, n_nodes, n_cores, u_max, sub=512):
        self.N = n_nodes
        self.C = n_cores
        self.NLOC = n_nodes // n_cores
        assert self.NLOC * n_cores == n_nodes
        self.NBLK = (self.NLOC + 127) // 128
        self.NLOC_PAD = self.NBLK * 128
        self.U = u_max  # compacted col-table rows (multiple of 128)
        self.SUB = sub  # edge sub-chunk (<=512, PSUM bank limit)


FULL = Cfg(50000, 8, 32768)


def _silu(nc, wrk, use_silu, out, xp, sub, bias, SUB):
    if use_silu:
        nc.scalar.activation(out[:, :sub], xp[:, :sub],
                             mybir.ActivationFunctionType.Silu, bias=bias[:])
    else:  # CoreSim lacks Silu: z*sigmoid(z)
        z = wrk.tile([128, SUB], BF16, tag="slz")
        nc.scalar.activation(z[:, :sub], xp[:, :sub],
                             mybir.ActivationFunctionType.Identity, bias=bias[:])
        sg = wrk.tile([128, SUB], BF16, tag="slg")
        nc.scalar.activation(sg[:, :sub], z[:, :sub],
                             mybir.ActivationFunctionType.Sigmoid)
        nc.vector.tensor_tensor(out[:, :sub], z[:, :sub], sg[:, :sub],
                                mybir.AluOpType.mult)


def _wrap_idx(idx):
    """int16 gather-index layout: [i%16, i//16], replicated 8x to 128 rows."""
    n = len(idx)
    assert n % 16 == 0
    w = idx.astype(np.int16).reshape(n // 16, 16).T
    return np.tile(w, (8, 1)).copy()


def build_graph(cfg, n_slots, use_silu=True, b_mode="full", lean=False):
    """Build the SPMD Bass graph. n_slots: per-block edge-slot counts
    (multiples of 128, identical across cores)."""
    nc = bacc.Bacc(None, target_bir_lowering=False, debug=False)
    NLOC_PAD, U, NBLK, SUB = cfg.NLOC_PAD, cfg.U, cfg.NBLK, cfg.SUB
    ES = int(sum(n_slots))  # total edge slots
    NSUB = ES // 128

    di = lambda name, shape, dt: nc.dram_tensor(name, shape, dt, kind="ExternalInput")
    # node-side uploads
    h_loc = di("h_loc", [NLOC_PAD, 128], BF16)
    h_u = di("h_u", [U, 128], BF16)
    vec_loc_T = di("vec_loc_T", [3, 128, NLOC_PAD], BF16)
    vec_T_u = di("vec_T_u", [3, 128, U], BF16)
    # edge-side uploads
    ucol_w = di("ucol_w", [128, ES // 16], I16)
    row_rel = di("row_rel", [128, NSUB], F32)
    xij_t = di("xij_t", [128, NSUB, 3], BF16)
    dense_bf = di("dense_bf", [48, ES], BF16)  # rows 0:32 d_ij, 32:48 edge_attr
    # weights
    wvp = di("wvp", [128, 384], BF16)
    w1p = di("w1p", [4, 128, 128], BF16)
    w2 = di("w2", [128, 128], BF16)
    w3 = di("w3", [128, 384], BF16)
    wop = di("wop", [128, 384], BF16)
    b1 = di("b1", [128, 1], F32)
    b2 = di("b2", [128, 1], F32)
    b3r = di("b3r", [1, 384], F32)
    bop = di("bop", [128, 3], F32)

    dh_o = nc.dram_tensor("dh", [cfg.NLOC, 128], F32, kind="ExternalOutput")
    dvec_o = nc.dram_tensor("dvec", [cfg.NLOC, 384], F32, kind="ExternalOutput")

    # internal DRAM gather tables
    # tab_A row: [h | v2_0 | v3_0 | v2_1 | v3_1 | v2_2 | v3_2] (7 x 128 bf16)
    tab_A = nc.dram_tensor("tab_A", [U, 896], BF16)
    tab_R = nc.dram_tensor("tab_R", [NLOC_PAD, 512], BF16)  # [h | vec1_v0..v2]

    with tile.TileContext(nc) as tc:
        with (
            tc.tile_pool(name="resident", bufs=1) as res,
            tc.tile_pool(name="stage", bufs=2 if lean else 3) as stg,
            tc.tile_pool(name="blk", bufs=1 if lean else 2) as blk,
            tc.tile_pool(name="work", bufs=2 if lean else 3) as wrk,
            tc.tile_pool(name="psA", bufs=2, space=bass.MemorySpace.PSUM) as psA,
            tc.tile_pool(name="psW", bufs=2, space=bass.MemorySpace.PSUM) as psW,
            tc.tile_pool(name="psM", bufs=2, space=bass.MemorySpace.PSUM) as psM,
            tc.tile_pool(name="psE", bufs=2, space=bass.MemorySpace.PSUM) as psE,
        ):
            # ---- one-time setup ----
            iota_i = res.tile([128, 128], I32)
            nc.gpsimd.iota(iota_i[:], [[1, 128]], channel_multiplier=0)
            iota_colf = res.tile([128, 128], F32)
            nc.vector.tensor_copy(iota_colf[:], iota_i[:])
            iota_p = res.tile([128, 1], I32)
            nc.gpsimd.iota(iota_p[:], [[1, 1]], channel_multiplier=1)
            iota_pf = res.tile([128, 1], F32)
            nc.vector.tensor_copy(iota_pf[:], iota_p[:])
            ident_bf = res.tile([128, 128], BF16)
            nc.vector.tensor_tensor(
                ident_bf[:], iota_colf[:], iota_pf[:].to_broadcast((128, 128)),
                mybir.AluOpType.is_equal)

            ones1 = res.tile([1, 128], F32)
            nc.vector.memset(ones1[:], 1.0)
            b3row = res.tile([1, 384], F32)
            nc.sync.dma_start(b3row[:], b3r[:])
            b3p = psM.tile([128, 512], F32, tag="m")
            nc.tensor.matmul(b3p[:, 0:384], ones1[:], b3row[:])
            b3_bc = res.tile([128, 384], F32)
            nc.scalar.copy(b3_bc[:], b3p[:, 0:384])

            # weights to SBUF
            wvp_s = res.tile([128, 384], BF16)
            nc.sync.dma_start(wvp_s[:], wvp[:])
            w1_s = res.tile([128, 4, 128], BF16)
            nc.sync.dma_start(w1_s[:], w1p[:].rearrange("k p f -> p k f"))
            w2_s = res.tile([128, 128], BF16)
            nc.sync.dma_start(w2_s[:], w2[:])
            w3_s = res.tile([128, 384], BF16)
            nc.sync.dma_start(w3_s[:], w3[:])
            wop_s = res.tile([128, 384], BF16)
            nc.sync.dma_start(wop_s[:], wop[:])
            b1_s = res.tile([128, 1], F32)
            nc.sync.dma_start(b1_s[:], b1[:])
            b2_s = res.tile([128, 1], F32)
            nc.sync.dma_start(b2_s[:], b2[:])
            bop_s = res.tile([128, 3], F32)
            nc.sync.dma_start(bop_s[:], bop[:])

            # resident edge metadata
            ucol_s = res.tile([128, ES // 16], I16)
            nc.sync.dma_start(ucol_s[:], ucol_w[:])
            rrel_s = res.tile([128, NSUB], F32)
            nc.sync.dma_start(rrel_s[:], row_rel[:])
            xij_s = res.tile([128, NSUB, 3], BF16)
            nc.sync.dma_start(xij_s[:], xij_t[:])

            # resident node data filled by phase A
            vdot_s = res.tile([128, cfg.NBLK, 128], BF16)
            v3_s = res.tile([128, cfg.NBLK, 384], BF16)

            # ---- phase A: local vecp, vec_dot, vec3, row table ----
            nc.sync.dma_start(
                tab_R[:].rearrange("n (t f) -> n t f", t=4)[:, 0, :], h_loc[:])
            for t in range(NLOC_PAD // 128):
                vlt = stg.tile([128, 3, 128], BF16)
                nc.sync.dma_start(
                    vlt[:],
                    vec_loc_T[:, :, 128 * t:128 * (t + 1)].rearrange(
                        "v p u -> p v u"))
                v1st = stg.tile([128, 3, 128], BF16)
                acc = None
                for v in range(3):
                    p = psM.tile([128, 512], F32, tag="m")
                    nc.tensor.matmul(p[:, 0:384], vlt[:, v, :], wvp_s[:])
                    nc.vector.tensor_copy(v1st[:, v, :], p[:, 0:128])
                    tm = wrk.tile([128, 128], F32)
                    nc.vector.tensor_tensor(tm[:], p[:, 128:256], v1st[:, v, :],
                                            mybir.AluOpType.mult)
                    if v == 0:
                        acc = tm
                    elif v == 1:
                        nc.vector.tensor_tensor(acc[:], acc[:], tm[:],
                                                mybir.AluOpType.add)
                    else:
                        nc.vector.tensor_tensor(vdot_s[:, t, :], acc[:], tm[:],
                                                mybir.AluOpType.add)
                    nc.scalar.copy(v3_s[:, t, 128 * v:128 * (v + 1)],
                                   p[:, 256:384])
                nc.sync.dma_start(
                    tab_R[128 * t:128 * (t + 1), 128:512], v1st[:])

            # ---- phase A2: compacted col tables ----
            nc.sync.dma_start(
                tab_A[:].rearrange("n (t f) -> n t f", t=7)[:, 0, :], h_u[:])
            for g in range(U // 512):  # groups of 4 u-tiles
                vtg = stg.tile([128, 3, 512], BF16)
                nc.sync.dma_start(
                    vtg[:],
                    vec_T_u[:, :, 512 * g:512 * (g + 1)].rearrange(
                        "v p u -> p v u"))
                stA = stg.tile([128, 4, 768], BF16)
                for j in range(4):
                    ut = 4 * g + j
                    for v in range(3):
                        p = psM.tile([128, 512], F32, tag="m")
                        nc.tensor.matmul(
                            p[:, 0:256], vtg[:, v, 128 * j:128 * (j + 1)],
                            wvp_s[:, 128:384])
                        nc.vector.tensor_copy(stA[:, j, 256 * v:256 * v + 128],
                                              p[:, 0:128])
                        nc.scalar.copy(stA[:, j, 256 * v + 128:256 * v + 256],
                                       p[:, 128:256])
                uslc = slice(512 * g, 512 * (g + 1))
                nc.sync.dma_start(
                    tab_A[uslc, 128:896].rearrange("(j p) f -> p j f", p=128),
                    stA[:])

            # ---- phase B: per node-block edge pipeline + fused epilogue ----
            ones_bf = res.tile([128, 128], BF16)
            nc.vector.memset(ones_bf[:], 1.0)
            zed = res.tile([128, 512], BF16)
            nc.vector.memset(zed[:], 0.0)
            if b_mode == "tables":
                for b in range(NBLK):
                    rows = min(128, cfg.NLOC - 128 * b)
                    t = wrk.tile([128, 64], F32, tag="tt")
                    nc.sync.dma_start(
                        t[:], tab_R[128 * b:128 * (b + 1), 0:128].bitcast(F32))
                    nc.sync.dma_start(dh_o[128 * b:128 * b + rows, 0:64],
                                      t[0:rows, :])
            s_off = 0  # slot offset (multiple of 128)
            for b in range(NBLK if b_mode != "tables" else 0):
                ns = int(n_slots[b])
                if ns == 0:
                    continue

                rb = blk.tile([128, 512], BF16, tag="rb")
                nc.sync.dma_start(rb[:], tab_R[128 * b:128 * (b + 1), :])
                agg = psA.tile([128, 512], F32)
                nc.tensor.matmul(agg[:], ones_bf[:], zed[:], start=True,
                                 stop=False, skip_group_check=True)
                nq = ns // 128
                for c0 in range(0, ns, SUB):
                    sub = min(SUB, ns - c0)
                    ia = (s_off + c0) // 16
                    ib = (s_off + c0 + sub) // 16
                    gA_t = blk.tile([128, 7 * SUB], BF16, tag="gA")
                    gA = gA_t[:, :7 * sub].rearrange("p (t s) -> p t s", t=7)
                    nc.gpsimd.dma_gather(gA, tab_A[:],
                                         ucol_s[:, ia:ib], sub, sub, 896,
                                         transpose=True)
                    # expand row-side features (h, vec1) via one-hot matmuls
                    rXP = wrk.tile([128, 4, SUB], BF16, tag="rXP")
                    Ssub = wrk.tile([128, 4, 128], BF16, tag="Ssub")
                    for ql in range(sub // 128):
                        q = (s_off + c0) // 128 + ql
                        nc.vector.tensor_tensor(
                            Ssub[:, ql, :],
                            rrel_s[:, q:q + 1].to_broadcast((128, 128)),
                            iota_colf[:], mybir.AluOpType.is_equal)
                        STp = psE.tile([128, 1024], BF16, tag="e")
                        nc.tensor.transpose(STp[:, 0:128], Ssub[:, ql, :],
                                            ident_bf[:])
                        STs = wrk.tile([128, 128], BF16, tag="STs")
                        nc.scalar.copy(STs[:], STp[:, 0:128])
                        psR = psE.tile([128, 512], F32, tag="e")
                        for t in range(4):
                            nc.tensor.matmul(psR[:, 128 * t:128 * (t + 1)],
                                             rb[:, 128 * t:128 * (t + 1)],
                                             STs[:], skip_group_check=True)
                        nc.scalar.copy(
                            rXP[:, :, 128 * ql:128 * (ql + 1)],
                            psR[:].rearrange("p (t f) -> p t f", t=4))
                    # cross = sum_v vec1row_v * vec2col_v  (feature-major)
                    cr = wrk.tile([128, SUB], BF16)
                    tt = wrk.tile([128, SUB], BF16)
                    nc.vector.tensor_tensor(
                        cr[:, :sub], rXP[:, 1, :sub], gA[:, 1, :sub],
                        mybir.AluOpType.mult)
                    nc.vector.tensor_tensor(
                        tt[:, :sub], rXP[:, 2, :sub], gA[:, 3, :sub],
                        mybir.AluOpType.mult)
                    nc.vector.tensor_tensor(cr[:, :sub], cr[:, :sub], tt[:, :sub],
                                            mybir.AluOpType.add)
                    nc.vector.tensor_tensor(
                        tt[:, :sub], rXP[:, 3, :sub], gA[:, 5, :sub],
                        mybir.AluOpType.mult)
                    nc.vector.tensor_tensor(cr[:, :sub], cr[:, :sub], tt[:, :sub],
                                            mybir.AluOpType.add)
                    # k3 = [d_ij(32) | edge_attr(16)], 48-partition matmul
                    k3 = wrk.tile([48, SUB], BF16)
                    nc.sync.dma_start(k3[:, :sub],
                                      dense_bf[:, s_off + c0:s_off + c0 + sub])
                    # L1
                    x1p = psW.tile([128, SUB], F32, tag="w")
                    nc.tensor.matmul(x1p[:, :sub], w1_s[:, 0, :],
                                     rXP[:, 0, :sub], start=True, stop=False)
                    nc.tensor.matmul(x1p[:, :sub], w1_s[:, 1, :],
                                     gA[:, 0, :sub], start=False, stop=False)
                    nc.tensor.matmul(x1p[:, :sub], w1_s[:, 2, :], cr[:, :sub],
                                     start=False, stop=False)
                    nc.tensor.matmul(x1p[:, :sub], w1_s[0:48, 3, :], k3[:, :sub],
                                     start=False, stop=True)
                    x1 = wrk.tile([128, SUB], BF16)
                    _silu(nc, wrk, use_silu, x1, x1p, sub, b1_s, SUB)
                    # L2
                    x2p = psW.tile([128, SUB], F32, tag="w")
                    nc.tensor.matmul(x2p[:, :sub], w2_s[:], x1[:, :sub])
                    x2 = wrk.tile([128, SUB], BF16)
                    _silu(nc, wrk, use_silu, x2, x2p, sub, b2_s, SUB)
                    # L3 flipped + aggregation per 128-edge subtile
                    for ql in range(sub // 128):
                        q = (s_off + c0) // 128 + ql
                        qb = (c0 // 128) + ql
                        mp = psM.tile([128, 512], F32, tag="m")
                        nc.tensor.matmul(mp[:, 0:384],
                                         x2[:, 128 * ql:128 * (ql + 1)], w3_s[:])
                        msg = wrk.tile([128, 384], BF16)
                        nc.vector.tensor_tensor(msg[:], mp[:, 0:384], b3_bc[:],
                                                mybir.AluOpType.add)
                        # vec3col to edge-major via TE transposes
                        v3p = psE.tile([128, 1024], BF16, tag="e")
                        for v in range(3):
                            nc.tensor.transpose(
                                v3p[:, 128 * v:128 * (v + 1)],
                                gA[:, 2 + 2 * v, 128 * ql:128 * (ql + 1)],
                                ident_bf[:])
                        v3e = wrk.tile([128, 384], BF16, tag="v3e")
                        nc.scalar.copy(v3e[:], v3p[:, 0:384])
                        # vec_msg = vec3col*m_v + x_ij*m_x  (edge-major)
                        t1 = wrk.tile([128, 3, 128], BF16)
                        nc.vector.tensor_tensor(
                            t1[:],
                            v3e[:].rearrange("p (v f) -> p v f", v=3),
                            msg[:, 128:256].rearrange("p (a f) -> p a f", a=1).to_broadcast(
                                (128, 3, 128)),
                            mybir.AluOpType.mult)
                        t2 = wrk.tile([128, 3, 128], BF16)
                        nc.vector.tensor_tensor(
                            t2[:],
                            xij_s[:, q, :].rearrange("p (v a) -> p v a", a=1).to_broadcast(
                                (128, 3, 128)),
                            msg[:, 256:384].rearrange("p (a f) -> p a f", a=1).to_broadcast(
                                (128, 3, 128)),
                            mybir.AluOpType.mult)
                        vm = wrk.tile([128, 3, 128], BF16)
                        nc.vector.tensor_tensor(vm[:], t1[:], t2[:],
                                                mybir.AluOpType.add)
                        # one-hot scatter
                        S = wrk.tile([128, 128], BF16)
                        nc.vector.tensor_tensor(
                            S[:], rrel_s[:, q:q + 1].to_broadcast((128, 128)),
                            iota_colf[:], mybir.AluOpType.is_equal)
                        first = False
                        last = qb == nq - 1
                        nc.tensor.matmul(agg[:, 0:128], Ssub[:, ql, :],
                                         msg[:, 0:128],
                                         start=first, stop=last,
                                         skip_group_check=True)
                        nc.tensor.matmul(
                            agg[:, 128:512], Ssub[:, ql, :],
                            vm[:].rearrange("p v f -> p (v f)"),
                            start=first, stop=last, skip_group_check=True)

                # ---- epilogue for block b ----
                hag = wrk.tile([128, 128], BF16)
                nc.vector.tensor_copy(hag[:], agg[:, 0:128])
                hagT_p = psE.tile([128, 1024], BF16, tag="e")
                nc.tensor.transpose(hagT_p[:, 0:128], hag[:], ident_bf[:])
                hagT = wrk.tile([128, 128], BF16)
                nc.scalar.copy(hagT[:], hagT_p[:, 0:128])
                oT = []
                for j in range(3):
                    op = psE.tile([128, 512], F32, tag="e")
                    nc.tensor.matmul(op[:, 0:128], wop_s[:, 128 * j:128 * (j + 1)],
                                     hagT[:])
                    osb = wrk.tile([128, 128], BF16)
                    nc.scalar.activation(osb[:], op[:, 0:128],
                                         mybir.ActivationFunctionType.Identity,
                                         bias=bop_s[:, j:j + 1])
                    otp = psE.tile([128, 1024], BF16, tag="e")
                    nc.tensor.transpose(otp[:, 0:128], osb[:], ident_bf[:])
                    ot = wrk.tile([128, 128], BF16)
                    nc.scalar.copy(ot[:], otp[:, 0:128])
                    oT.append(ot)
                rows = min(128, cfg.NLOC - 128 * b)
                dht = wrk.tile([128, 128], F32)
                nc.vector.tensor_tensor(dht[:], vdot_s[:, b, :], oT[1][:],
                                        mybir.AluOpType.mult)
                nc.vector.tensor_tensor(dht[:], dht[:], oT[2][:],
                                        mybir.AluOpType.add)
                nc.sync.dma_start(dh_o[128 * b:128 * b + rows, :],
                                  dht[0:rows, :])
                vma = wrk.tile([128, 3, 128], F32)
                nc.vector.tensor_tensor(
                    vma[:], v3_s[:, b, :].rearrange("p (v f) -> p v f", v=3),
                    oT[0][:].rearrange("p (a f) -> p a f", a=1).to_broadcast(
                        (128, 3, 128)),
                    mybir.AluOpType.mult)
                dvt = wrk.tile([128, 3, 128], F32)
                nc.vector.tensor_tensor(
                    dvt[:], vma[:],
                    agg[:, 128:512].rearrange("p (v f) -> p v f", v=3),
                    mybir.AluOpType.add)
                nc.sync.dma_start(dvec_o[128 * b:128 * b + rows, :],
                                  dvt[0:rows, :, :].rearrange("p v f -> p (v f)"))
                s_off += ns

    nc.compile()
    return nc


def host_prep(cfg, h, vec, coord, edge_index, edge_attr,
              Wvp, W1, b1, W2, b2, W3, b3, Wop, bop):
    """Shard + lay out all inputs. Returns (n_slots, in_maps)."""
    C, NLOC, NBLK, U = cfg.C, cfg.NLOC, cfg.NBLK, cfg.U
    bf = ml_dtypes.bfloat16
    row = np.asarray(edge_index[0], np.int64)
    col = np.asarray(edge_index[1], np.int64)
    E = row.shape[0]
    h = np.asarray(h, np.float32)
    vec = np.asarray(vec, np.float32)
    coord = np.asarray(coord, np.float32)
    edge_attr = np.asarray(edge_attr, np.float32)

    x_ij = coord[row] - coord[col]
    d = np.sqrt((x_ij * x_ij).sum(-1) + EPS)
    offs = np.linspace(0.0, CUTOFF, NRBF, dtype=np.float32)
    coeff = np.float32(-0.5 / (offs[1] - offs[0]) ** 2)
    d_ij = np.exp(coeff * (d[:, None] - offs[None, :]) ** 2).astype(np.float32)

    core = row // NLOC
    lrow = (row - core * NLOC).astype(np.int64)
    blk = lrow // 128

    # per-core, per-block edge lists
    per = []
    cnt = np.zeros((C, NBLK), np.int64)
    for c in range(C):
        m = np.nonzero(core == c)[0]
        order = np.argsort(lrow[m], kind="stable")
        e = m[order]
        per.append(e)
        cb = np.bincount(blk[e], minlength=NBLK)
        cnt[c] = cb
    n_slots = (np.ceil(cnt.max(axis=0) / 128).astype(np.int64) * 128)
    n_slots = np.maximum(n_slots, 128)
    ES = int(n_slots.sum())
    NSUB = ES // 128
    starts = np.concatenate([[0], np.cumsum(n_slots)[:-1]])

    # MLP weight repack: k-tiles [h_row, h_col, cross, (d_ij|edge_attr|0)]
    W1 = np.asarray(W1, np.float32)
    w1p = np.zeros((4, 128, 128), np.float32)
    w1p[0] = W1[0:128]
    w1p[1] = W1[128:256]
    w1p[2] = W1[288:416]
    w1p[3, 0:32] = W1[256:288]
    w1p[3, 32:48] = W1[416:432]

    in_maps = []
    for c in range(C):
        e = per[c]
        lr = lrow[e]
        bl = blk[e]
        # slot positions
        pos = np.empty(len(e), np.int64)
        off = np.zeros(NBLK, np.int64)
        # edges are sorted by lrow hence by block; place sequentially per block
        for b in range(NBLK):
            k = np.nonzero(bl == b)[0]
            pos[k] = starts[b] + np.arange(len(k))
        ucols, uinv = np.unique(col[e], return_inverse=True)
        nu = len(ucols)
        assert nu <= U, f"core {c}: {nu} unique cols > {U}"

        ucol_a = np.zeros(ES, np.int64)
        urow_a = np.zeros(ES, np.int64)
        rrel_a = np.full(ES, -1.0, np.float32)
        xij_a = np.zeros((ES, 3), np.float32)
        dense_a = np.zeros((48, ES), np.float32)
        ucol_a[pos] = uinv
        urow_a[pos] = lr
        rrel_a[pos] = (lr - 128 * bl).astype(np.float32)
        xij_a[pos] = x_ij[e]
        dense_a[0:32, pos] = d_ij[e].T
        dense_a[32:48, pos] = edge_attr[e].T

        nlp = cfg.NLOC_PAD
        h_loc = np.zeros((nlp, 128), np.float32)
        h_loc[0:NLOC] = h[c * NLOC:(c + 1) * NLOC]
        vl = np.zeros((3, 128, nlp), np.float32)
        vl[:, :, 0:NLOC] = vec[c * NLOC:(c + 1) * NLOC].transpose(1, 2, 0)
        h_uu = np.zeros((U, 128), np.float32)
        h_uu[0:nu] = h[ucols]
        vtu = np.zeros((3, 128, U), np.float32)
        vtu[:, :, 0:nu] = vec[ucols].transpose(1, 2, 0)

        in_maps.append({
            "h_loc": h_loc.astype(bf),
            "h_u": h_uu.astype(bf),
            "vec_loc_T": vl.astype(bf),
            "vec_T_u": vtu.astype(bf),
            "ucol_w": _wrap_idx(ucol_a),
            "urow_w": _wrap_idx(urow_a),
            "row_rel": rrel_a.reshape(NSUB, 128).T.copy(),
            "xij_t": xij_a.reshape(NSUB, 128, 3).transpose(1, 0, 2).astype(bf),
            "dense_bf": dense_a.astype(bf),
            "wvp": np.asarray(Wvp, np.float32).astype(bf),
            "w1p": w1p.astype(bf),
            "w2": np.asarray(W2, np.float32).astype(bf),
            "w3": np.asarray(W3, np.float32).astype(bf),
            "wop": np.asarray(Wop, np.float32).astype(bf),
            "b1": np.asarray(b1, np.float32).reshape(128, 1),
            "b2": np.asarray(b2, np.float32).reshape(128, 1),
            "b3r": np.asarray(b3, np.float32).reshape(1, 384),
            "bop": np.asarray(bop, np.float32).reshape(3, 128).T.copy(),
        })
    return n_slots, in_maps


_CACHE = {}


def _get_graph(cfg, n_slots):
    key = (cfg.N, cfg.C, tuple(int(x) for x in n_slots))
    if key not in _CACHE:
        _CACHE[key] = build_graph(cfg, n_slots)
    return _CACHE[key]


def kernel(h, vec, coord, edge_index, edge_attr,
           Wvp, W1, b1, W2, b2, W3, b3, Wop, bop):
    cfg = FULL
    n_slots, in_maps = host_prep(cfg, h, vec, coord, edge_index, edge_attr,
                                 Wvp, W1, b1, W2, b2, W3, b3, Wop, bop)
    nc = _get_graph(cfg, n_slots)
    trace = bool(int(os.environ.get("BASS_KERNEL_TRACE", "0")))
    res = run_bass_kernel_spmd(nc, in_maps, list(range(cfg.C)), trace=trace)
    kernel.last_exec_time_ns = res.exec_time_ns
    kernel.last_results = res
    dh = np.concatenate([res.results[c]["dh"] for c in range(cfg.C)], axis=0)
    dvec = np.concatenate([res.results[c]["dvec"] for c in range(cfg.C)],
                          axis=0).reshape(cfg.N, 3, 128)
    return dh, dvec


# revision 19
# speedup vs baseline: 1.3345x; 1.0716x over previous
"""Trainium2 Bass kernel for the PaiNN-style GNN message-passing layer.

Strategy (8 NeuronCores, SPMD, no collectives):
- Node rows are block-sharded: core c owns rows [c*NLOC, (c+1)*NLOC).
- Each edge is assigned to the core owning its destination (row) node, so
  all scatter-adds are core-local.
- Per-core compacted endpoint tables (unique cols < 32768) make gather
  indices fit the int16 DMA-gather index format.
- Edge MLP runs in bf16, features-on-partitions; the final MLP layer is
  computed "flipped" (activations as the stationary operand) so messages
  come out edge-major, ready for aggregation without transposes.
- Aggregation is an exact f32 one-hot matmul into a block-persistent PSUM
  tile (one 128-row node block at a time), fused with the output epilogue.
"""

import os
import numpy as np
import ml_dtypes

from concourse import bacc, bass, tile, mybir
from concourse.bass_utils import run_bass_kernel_spmd

BF16 = mybir.dt.bfloat16
F32 = mybir.dt.float32
I16 = mybir.dt.int16
I32 = mybir.dt.int32

HID = 128
NRBF = 32
EDGE = 16
CUTOFF = 5.0
EPS = 1e-8


class Cfg:
    def __init__(self, n_nodes, n_cores, u_max, sub=512):
        self.N = n_nodes
        self.C = n_cores
        self.NLOC = n_nodes // n_cores
        assert self.NLOC * n_cores == n_nodes
        self.NBLK = (self.NLOC + 127) // 128
        self.NLOC_PAD = self.NBLK * 128
        self.U = u_max  # compacted col-table rows (multiple of 128)
        self.SUB = sub  # edge sub-chunk (<=512, PSUM bank limit)


FULL = Cfg(50000, 8, 32768)


def _silu(nc, wrk, use_silu, out, xp, sub, bias, SUB):
    if use_silu:
        nc.scalar.activation(out[:, :sub], xp[:, :sub],
                             mybir.ActivationFunctionType.Silu, bias=bias[:])
    else:  # CoreSim lacks Silu: z*sigmoid(z)
        z = wrk.tile([128, SUB], BF16, tag="slz")
        nc.scalar.activation(z[:, :sub], xp[:, :sub],
                             mybir.ActivationFunctionType.Identity, bias=bias[:])
        sg = wrk.tile([128, SUB], BF16, tag="slg")
        nc.scalar.activation(sg[:, :sub], z[:, :sub],
                             mybir.ActivationFunctionType.Sigmoid)
        nc.vector.tensor_tensor(out[:, :sub], z[:, :sub], sg[:, :sub],
                                mybir.AluOpType.mult)


def _wrap_idx(idx):
    """int16 gather-index layout: [i%16, i//16], replicated 8x to 128 rows."""
    n = len(idx)
    assert n % 16 == 0
    w = idx.astype(np.int16).reshape(n // 16, 16).T
    return np.tile(w, (8, 1)).copy()


def build_graph(cfg, n_slots, use_silu=True, b_mode="full", lean=False):
    """Build the SPMD Bass graph. n_slots: per-block edge-slot counts
    (multiples of 128, identical across cores)."""
    nc = bacc.Bacc(None, target_bir_lowering=False, debug=False)
    NLOC_PAD, U, NBLK, SUB = cfg.NLOC_PAD, cfg.U, cfg.NBLK, cfg.SUB
    ES = int(sum(n_slots))  # total edge slots
    NSUB = ES // 128

    di = lambda name, shape, dt: nc.dram_tensor(name, shape, dt, kind="ExternalInput")
    # node-side uploads
    h_loc = di("h_loc", [NLOC_PAD, 128], BF16)
    h_u = di("h_u", [U, 128], BF16)
    vec_loc_T = di("vec_loc_T", [3, 128, NLOC_PAD], BF16)
    vec_T_u = di("vec_T_u", [3, 128, U], BF16)
    # edge-side uploads
    ucol_w = di("ucol_w", [128, ES // 16], I16)
    urow_w = di("urow_w", [128, ES // 16], I16)
    row_rel = di("row_rel", [128, NSUB], BF16)
    xij_t = di("xij_t", [128, NSUB, 3], BF16)
    dense_bf = di("dense_bf", [48, ES], BF16)  # rows 0:32 d_ij, 32:48 edge_attr
    # weights
    wvp = di("wvp", [128, 384], BF16)
    w1p = di("w1p", [4, 128, 128], BF16)
    w2 = di("w2", [128, 128], BF16)
    w3 = di("w3", [128, 384], BF16)
    wop = di("wop", [128, 384], BF16)
    b1 = di("b1", [128, 1], F32)
    b2 = di("b2", [128, 1], F32)
    b3r = di("b3r", [1, 384], F32)
    bop = di("bop", [128, 3], F32)

    dh_o = nc.dram_tensor("dh", [cfg.NLOC, 128], F32, kind="ExternalOutput")
    dvec_o = nc.dram_tensor("dvec", [cfg.NLOC, 384], F32, kind="ExternalOutput")

    # internal DRAM gather tables
    # tab_A row: [h | v2_0 | v3_0 | v2_1 | v3_1 | v2_2 | v3_2] (7 x 128 bf16)
    tab_A = nc.dram_tensor("tab_A", [U, 896], BF16)
    tab_R = nc.dram_tensor("tab_R", [NLOC_PAD, 512], BF16)  # [h | vec1_v0..v2]

    with tile.TileContext(nc) as tc:
        with (
            tc.tile_pool(name="resident", bufs=1) as res,
            tc.tile_pool(name="stage", bufs=2 if lean else 3) as stg,
            tc.tile_pool(name="blk", bufs=1 if lean else 2) as blk,
            tc.tile_pool(name="work", bufs=2 if lean else 3) as wrk,
            tc.tile_pool(name="psA", bufs=2, space=bass.MemorySpace.PSUM) as psA,
            tc.tile_pool(name="psW", bufs=1, space=bass.MemorySpace.PSUM) as psW,
            tc.tile_pool(name="psM", bufs=2, space=bass.MemorySpace.PSUM) as psM,
            tc.tile_pool(name="psE", bufs=3, space=bass.MemorySpace.PSUM) as psE,
        ):
            # ---- one-time setup ----
            iota_i = res.tile([128, 128], I32)
            nc.gpsimd.iota(iota_i[:], [[1, 128]], channel_multiplier=0)
            iota_colf = res.tile([128, 128], F32)
            nc.vector.tensor_copy(iota_colf[:], iota_i[:])
            iota_p = res.tile([128, 1], I32)
            nc.gpsimd.iota(iota_p[:], [[1, 1]], channel_multiplier=1)
            iota_pf = res.tile([128, 1], F32)
            nc.vector.tensor_copy(iota_pf[:], iota_p[:])
            iota_cbf = res.tile([128, 128], BF16)
            nc.vector.tensor_copy(iota_cbf[:], iota_colf[:])
            ident_bf = res.tile([128, 128], BF16)
            nc.vector.tensor_tensor(
                ident_bf[:], iota_colf[:], iota_pf[:].to_broadcast((128, 128)),
                mybir.AluOpType.is_equal)

            ones1 = res.tile([1, 128], F32)
            nc.vector.memset(ones1[:], 1.0)
            b3row = res.tile([1, 384], F32)
            nc.sync.dma_start(b3row[:], b3r[:])
            b3p = psM.tile([128, 512], F32, tag="m")
            nc.tensor.matmul(b3p[:, 0:384], ones1[:], b3row[:])
            b3_bc = res.tile([128, 384], F32)
            nc.scalar.copy(b3_bc[:], b3p[:, 0:384])

            # weights to SBUF
            wvp_s = res.tile([128, 384], BF16)
            nc.sync.dma_start(wvp_s[:], wvp[:])
            w1_s = res.tile([128, 4, 128], BF16)
            nc.sync.dma_start(w1_s[:], w1p[:].rearrange("k p f -> p k f"))
            w2_s = res.tile([128, 128], BF16)
            nc.sync.dma_start(w2_s[:], w2[:])
            w3_s = res.tile([128, 384], BF16)
            nc.sync.dma_start(w3_s[:], w3[:])
            wop_s = res.tile([128, 384], BF16)
            nc.sync.dma_start(wop_s[:], wop[:])
            b1_s = res.tile([128, 1], F32)
            nc.sync.dma_start(b1_s[:], b1[:])
            b2_s = res.tile([128, 1], F32)
            nc.sync.dma_start(b2_s[:], b2[:])
            bop_s = res.tile([128, 3], F32)
            nc.sync.dma_start(bop_s[:], bop[:])

            # resident edge metadata
            ucol_s = res.tile([128, ES // 16], I16)
            nc.sync.dma_start(ucol_s[:], ucol_w[:])
            urow_s = res.tile([128, ES // 16], I16)
            nc.sync.dma_start(urow_s[:], urow_w[:])
            rrel_s = res.tile([128, NSUB], BF16)
            nc.sync.dma_start(rrel_s[:], row_rel[:])
            xij_s = res.tile([128, NSUB, 3], BF16)
            nc.sync.dma_start(xij_s[:], xij_t[:])

            # resident node data filled by phase A
            vdot_s = res.tile([128, cfg.NBLK, 128], BF16)
            v3_s = res.tile([128, cfg.NBLK, 384], BF16)

            # ---- phase A: local vecp, vec_dot, vec3, row table ----
            nc.sync.dma_start(
                tab_R[:].rearrange("n (t f) -> n t f", t=4)[:, 0, :], h_loc[:])
            for t in range(NLOC_PAD // 128):
                vlt = stg.tile([128, 3, 128], BF16)
                nc.sync.dma_start(
                    vlt[:],
                    vec_loc_T[:, :, 128 * t:128 * (t + 1)].rearrange(
                        "v p u -> p v u"))
                v1st = stg.tile([128, 3, 128], BF16)
                acc = None
                for v in range(3):
                    p = psM.tile([128, 512], F32, tag="m")
                    nc.tensor.matmul(p[:, 0:384], vlt[:, v, :], wvp_s[:])
                    nc.vector.tensor_copy(v1st[:, v, :], p[:, 0:128])
                    tm = wrk.tile([128, 128], F32)
                    nc.vector.tensor_tensor(tm[:], p[:, 128:256], v1st[:, v, :],
                                            mybir.AluOpType.mult)
                    if v == 0:
                        acc = tm
                    elif v == 1:
                        nc.vector.tensor_tensor(acc[:], acc[:], tm[:],
                                                mybir.AluOpType.add)
                    else:
                        nc.vector.tensor_tensor(vdot_s[:, t, :], acc[:], tm[:],
                                                mybir.AluOpType.add)
                    nc.scalar.copy(v3_s[:, t, 128 * v:128 * (v + 1)],
                                   p[:, 256:384])
                nc.sync.dma_start(
                    tab_R[128 * t:128 * (t + 1), 128:512], v1st[:])

            # ---- phase A2: compacted col tables ----
            nc.sync.dma_start(
                tab_A[:].rearrange("n (t f) -> n t f", t=7)[:, 0, :], h_u[:])
            for g in range(U // 512):  # groups of 4 u-tiles
                vtg = stg.tile([128, 3, 512], BF16)
                nc.sync.dma_start(
                    vtg[:],
                    vec_T_u[:, :, 512 * g:512 * (g + 1)].rearrange(
                        "v p u -> p v u"))
                stA = stg.tile([128, 4, 768], BF16)
                for j in range(4):
                    ut = 4 * g + j
                    for v in range(3):
                        p = psM.tile([128, 512], F32, tag="m")
                        nc.tensor.matmul(
                            p[:, 0:256], vtg[:, v, 128 * j:128 * (j + 1)],
                            wvp_s[:, 128:384])
                        nc.scalar.copy(stA[:, j, 256 * v:256 * v + 128],
                                       p[:, 0:128])
                        nc.scalar.copy(stA[:, j, 256 * v + 128:256 * v + 256],
                                       p[:, 128:256])
                uslc = slice(512 * g, 512 * (g + 1))
                nc.sync.dma_start(
                    tab_A[uslc, 128:896].rearrange("(j p) f -> p j f", p=128),
                    stA[:])

            # ---- phase B: per node-block edge pipeline + fused epilogue ----
            ones_bf = res.tile([128, 128], BF16)
            nc.vector.memset(ones_bf[:], 1.0)
            zed = res.tile([128, 512], BF16)
            nc.vector.memset(zed[:], 0.0)
            if b_mode == "tables":
                for b in range(NBLK):
                    rows = min(128, cfg.NLOC - 128 * b)
                    t = wrk.tile([128, 64], F32, tag="tt")
                    nc.sync.dma_start(
                        t[:], tab_R[128 * b:128 * (b + 1), 0:128].bitcast(F32))
                    nc.sync.dma_start(dh_o[128 * b:128 * b + rows, 0:64],
                                      t[0:rows, :])
            s_off = 0  # slot offset (multiple of 128)
            for b in range(NBLK if b_mode != "tables" else 0):
                ns = int(n_slots[b])
                if ns == 0:
                    continue

                agg = psA.tile([128, 512], F32)
                nc.tensor.matmul(agg[:], ones_bf[:], zed[:], start=True,
                                 stop=False, skip_group_check=True)
                nq = ns // 128
                for c0 in range(0, ns, SUB):
                    sub = min(SUB, ns - c0)
                    ia = (s_off + c0) // 16
                    ib = (s_off + c0 + sub) // 16
                    gA_t = blk.tile([128, 7 * SUB], BF16, tag="gA")
                    gA = gA_t[:, :7 * sub].rearrange("p (t s) -> p t s", t=7)
                    nc.gpsimd.dma_gather(gA, tab_A[:],
                                         ucol_s[:, ia:ib], sub, sub, 896,
                                         transpose=True)
                    gR_t = blk.tile([128, 4 * SUB], BF16, tag="gR")
                    gR = gR_t[:, :4 * sub].rearrange("p (t s) -> p t s", t=4)
                    nc.gpsimd.dma_gather(gR, tab_R[:],
                                         urow_s[:, ia:ib], sub, sub, 512,
                                         transpose=True)
                    Ssub = wrk.tile([128, 4, 128], BF16, tag="Ssub")
                    for ql in range(sub // 128):
                        q = (s_off + c0) // 128 + ql
                        nc.vector.tensor_tensor(
                            Ssub[:, ql, :],
                            rrel_s[:, q:q + 1].to_broadcast((128, 128)),
                            iota_cbf[:], mybir.AluOpType.is_equal)
                    # cross = sum_v vec1row_v * vec2col_v  (feature-major)
                    cr = wrk.tile([128, SUB], BF16)
                    tt = wrk.tile([128, SUB], BF16)
                    nc.vector.tensor_tensor(
                        cr[:, :sub], gR[:, 1, :sub], gA[:, 1, :sub],
                        mybir.AluOpType.mult)
                    nc.vector.tensor_tensor(
                        tt[:, :sub], gR[:, 2, :sub], gA[:, 3, :sub],
                        mybir.AluOpType.mult)
                    nc.vector.tensor_tensor(cr[:, :sub], cr[:, :sub], tt[:, :sub],
                                            mybir.AluOpType.add)
                    nc.vector.tensor_tensor(
                        tt[:, :sub], gR[:, 3, :sub], gA[:, 5, :sub],
                        mybir.AluOpType.mult)
                    nc.vector.tensor_tensor(cr[:, :sub], cr[:, :sub], tt[:, :sub],
                                            mybir.AluOpType.add)
                    # k3 = [d_ij(32) | edge_attr(16)], 48-partition matmul
                    k3 = wrk.tile([48, SUB], BF16)
                    nc.sync.dma_start(k3[:, :sub],
                                      dense_bf[:, s_off + c0:s_off + c0 + sub])
                    # L1
                    x1p = psW.tile([128, SUB], F32, tag="w")
                    nc.tensor.matmul(x1p[:, :sub], w1_s[:, 0, :],
                                     gR[:, 0, :sub], start=True, stop=False)
                    nc.tensor.matmul(x1p[:, :sub], w1_s[:, 1, :],
                                     gA[:, 0, :sub], start=False, stop=False)
                    nc.tensor.matmul(x1p[:, :sub], w1_s[:, 2, :], cr[:, :sub],
                                     start=False, stop=False)
                    nc.tensor.matmul(x1p[:, :sub], w1_s[0:48, 3, :], k3[:, :sub],
                                     start=False, stop=True)
                    x1 = wrk.tile([128, SUB], BF16)
                    _silu(nc, wrk, use_silu, x1, x1p, sub, b1_s, SUB)
                    # L2
                    x2p = psW.tile([128, SUB], F32, tag="w")
                    nc.tensor.matmul(x2p[:, :sub], w2_s[:], x1[:, :sub])
                    x2 = wrk.tile([128, SUB], BF16)
                    _silu(nc, wrk, use_silu, x2, x2p, sub, b2_s, SUB)
                    # L3 flipped + aggregation per 128-edge subtile
                    for ql in range(sub // 128):
                        q = (s_off + c0) // 128 + ql
                        qb = (c0 // 128) + ql
                        mp = psM.tile([128, 512], F32, tag="m")
                        nc.tensor.matmul(mp[:, 0:384],
                                         x2[:, 128 * ql:128 * (ql + 1)], w3_s[:])
                        msg = wrk.tile([128, 384], BF16)
                        nc.vector.tensor_tensor(msg[:], mp[:, 0:384], b3_bc[:],
                                                mybir.AluOpType.add)
                        # vec3col to edge-major via TE transposes
                        v3p = psE.tile([128, 1024], BF16, tag="e")
                        for v in range(3):
                            nc.tensor.transpose(
                                v3p[:, 128 * v:128 * (v + 1)],
                                gA[:, 2 + 2 * v, 128 * ql:128 * (ql + 1)],
                                ident_bf[:])
                        v3e = wrk.tile([128, 384], BF16, tag="v3e")
                        nc.scalar.copy(v3e[:], v3p[:, 0:384])
                        # vec_msg = vec3col*m_v + x_ij*m_x  (edge-major)
                        t1 = wrk.tile([128, 3, 128], BF16)
                        nc.vector.tensor_tensor(
                            t1[:],
                            v3e[:].rearrange("p (v f) -> p v f", v=3),
                            msg[:, 128:256].rearrange("p (a f) -> p a f", a=1).to_broadcast(
                                (128, 3, 128)),
                            mybir.AluOpType.mult)
                        t2 = wrk.tile([128, 3, 128], BF16)
                        nc.vector.tensor_tensor(
                            t2[:],
                            xij_s[:, q, :].rearrange("p (v a) -> p v a", a=1).to_broadcast(
                                (128, 3, 128)),
                            msg[:, 256:384].rearrange("p (a f) -> p a f", a=1).to_broadcast(
                                (128, 3, 128)),
                            mybir.AluOpType.mult)
                        vm = wrk.tile([128, 3, 128], BF16)
                        nc.vector.tensor_tensor(vm[:], t1[:], t2[:],
                                                mybir.AluOpType.add)
                        # one-hot scatter
                        S = wrk.tile([128, 128], BF16)
                        nc.vector.tensor_tensor(
                            S[:], rrel_s[:, q:q + 1].to_broadcast((128, 128)),
                            iota_colf[:], mybir.AluOpType.is_equal)
                        first = False
                        last = qb == nq - 1
                        nc.tensor.matmul(agg[:, 0:128], Ssub[:, ql, :],
                                         msg[:, 0:128],
                                         start=first, stop=last,
                                         skip_group_check=True)
                        nc.tensor.matmul(
                            agg[:, 128:512], Ssub[:, ql, :],
                            vm[:].rearrange("p v f -> p (v f)"),
                            start=first, stop=last, skip_group_check=True)

                # ---- epilogue for block b ----
                hag = wrk.tile([128, 128], BF16)
                nc.vector.tensor_copy(hag[:], agg[:, 0:128])
                hagT_p = psE.tile([128, 1024], BF16, tag="e")
                nc.tensor.transpose(hagT_p[:, 0:128], hag[:], ident_bf[:])
                hagT = wrk.tile([128, 128], BF16)
                nc.scalar.copy(hagT[:], hagT_p[:, 0:128])
                oT = []
                for j in range(3):
                    op = psE.tile([128, 512], F32, tag="e")
                    nc.tensor.matmul(op[:, 0:128], wop_s[:, 128 * j:128 * (j + 1)],
                                     hagT[:])
                    osb = wrk.tile([128, 128], BF16)
                    nc.scalar.activation(osb[:], op[:, 0:128],
                                         mybir.ActivationFunctionType.Identity,
                                         bias=bop_s[:, j:j + 1])
                    otp = psE.tile([128, 1024], BF16, tag="e")
                    nc.tensor.transpose(otp[:, 0:128], osb[:], ident_bf[:])
                    ot = wrk.tile([128, 128], BF16)
                    nc.scalar.copy(ot[:], otp[:, 0:128])
                    oT.append(ot)
                rows = min(128, cfg.NLOC - 128 * b)
                dht = wrk.tile([128, 128], F32)
                nc.vector.tensor_tensor(dht[:], vdot_s[:, b, :], oT[1][:],
                                        mybir.AluOpType.mult)
                nc.vector.tensor_tensor(dht[:], dht[:], oT[2][:],
                                        mybir.AluOpType.add)
                nc.sync.dma_start(dh_o[128 * b:128 * b + rows, :],
                                  dht[0:rows, :])
                vma = wrk.tile([128, 3, 128], F32)
                nc.vector.tensor_tensor(
                    vma[:], v3_s[:, b, :].rearrange("p (v f) -> p v f", v=3),
                    oT[0][:].rearrange("p (a f) -> p a f", a=1).to_broadcast(
                        (128, 3, 128)),
                    mybir.AluOpType.mult)
                dvt = wrk.tile([128, 3, 128], F32)
                nc.vector.tensor_tensor(
                    dvt[:], vma[:],
                    agg[:, 128:512].rearrange("p (v f) -> p v f", v=3),
                    mybir.AluOpType.add)
                nc.sync.dma_start(dvec_o[128 * b:128 * b + rows, :],
                                  dvt[0:rows, :, :].rearrange("p v f -> p (v f)"))
                s_off += ns

    nc.compile()
    return nc


def host_prep(cfg, h, vec, coord, edge_index, edge_attr,
              Wvp, W1, b1, W2, b2, W3, b3, Wop, bop):
    """Shard + lay out all inputs. Returns (n_slots, in_maps)."""
    C, NLOC, NBLK, U = cfg.C, cfg.NLOC, cfg.NBLK, cfg.U
    bf = ml_dtypes.bfloat16
    row = np.asarray(edge_index[0], np.int64)
    col = np.asarray(edge_index[1], np.int64)
    E = row.shape[0]
    h = np.asarray(h, np.float32)
    vec = np.asarray(vec, np.float32)
    coord = np.asarray(coord, np.float32)
    edge_attr = np.asarray(edge_attr, np.float32)

    x_ij = coord[row] - coord[col]
    d = np.sqrt((x_ij * x_ij).sum(-1) + EPS)
    offs = np.linspace(0.0, CUTOFF, NRBF, dtype=np.float32)
    coeff = np.float32(-0.5 / (offs[1] - offs[0]) ** 2)
    d_ij = np.exp(coeff * (d[:, None] - offs[None, :]) ** 2).astype(np.float32)

    core = row // NLOC
    lrow = (row - core * NLOC).astype(np.int64)
    blk = lrow // 128

    # per-core, per-block edge lists
    per = []
    cnt = np.zeros((C, NBLK), np.int64)
    for c in range(C):
        m = np.nonzero(core == c)[0]
        order = np.argsort(lrow[m], kind="stable")
        e = m[order]
        per.append(e)
        cb = np.bincount(blk[e], minlength=NBLK)
        cnt[c] = cb
    n_slots = (np.ceil(cnt.max(axis=0) / 128).astype(np.int64) * 128)
    n_slots = np.maximum(n_slots, 128)
    ES = int(n_slots.sum())
    NSUB = ES // 128
    starts = np.concatenate([[0], np.cumsum(n_slots)[:-1]])

    # MLP weight repack: k-tiles [h_row, h_col, cross, (d_ij|edge_attr|0)]
    W1 = np.asarray(W1, np.float32)
    w1p = np.zeros((4, 128, 128), np.float32)
    w1p[0] = W1[0:128]
    w1p[1] = W1[128:256]
    w1p[2] = W1[288:416]
    w1p[3, 0:32] = W1[256:288]
    w1p[3, 32:48] = W1[416:432]

    in_maps = []
    for c in range(C):
        e = per[c]
        lr = lrow[e]
        bl = blk[e]
        # slot positions
        pos = np.empty(len(e), np.int64)
        off = np.zeros(NBLK, np.int64)
        # edges are sorted by lrow hence by block; place sequentially per block
        for b in range(NBLK):
            k = np.nonzero(bl == b)[0]
            pos[k] = starts[b] + np.arange(len(k))
        ucols, uinv = np.unique(col[e], return_inverse=True)
        nu = len(ucols)
        assert nu <= U, f"core {c}: {nu} unique cols > {U}"

        ucol_a = np.zeros(ES, np.int64)
        urow_a = np.zeros(ES, np.int64)
        rrel_a = np.full(ES, -1.0, np.float32)
        xij_a = np.zeros((ES, 3), np.float32)
        dense_a = np.zeros((48, ES), np.float32)
        ucol_a[pos] = uinv
        urow_a[pos] = lr
        rrel_a[pos] = (lr - 128 * bl).astype(np.float32)
        xij_a[pos] = x_ij[e]
        dense_a[0:32, pos] = d_ij[e].T
        dense_a[32:48, pos] = edge_attr[e].T

        nlp = cfg.NLOC_PAD
        h_loc = np.zeros((nlp, 128), np.float32)
        h_loc[0:NLOC] = h[c * NLOC:(c + 1) * NLOC]
        vl = np.zeros((3, 128, nlp), np.float32)
        vl[:, :, 0:NLOC] = vec[c * NLOC:(c + 1) * NLOC].transpose(1, 2, 0)
        h_uu = np.zeros((U, 128), np.float32)
        h_uu[0:nu] = h[ucols]
        vtu = np.zeros((3, 128, U), np.float32)
        vtu[:, :, 0:nu] = vec[ucols].transpose(1, 2, 0)

        in_maps.append({
            "h_loc": h_loc.astype(bf),
            "h_u": h_uu.astype(bf),
            "vec_loc_T": vl.astype(bf),
            "vec_T_u": vtu.astype(bf),
            "ucol_w": _wrap_idx(ucol_a),
            "urow_w": _wrap_idx(urow_a),
            "row_rel": rrel_a.reshape(NSUB, 128).T.astype(bf),
            "xij_t": xij_a.reshape(NSUB, 128, 3).transpose(1, 0, 2).astype(bf),
            "dense_bf": dense_a.astype(bf),
            "wvp": np.asarray(Wvp, np.float32).astype(bf),
            "w1p": w1p.astype(bf),
            "w2": np.asarray(W2, np.float32).astype(bf),
            "w3": np.asarray(W3, np.float32).astype(bf),
            "wop": np.asarray(Wop, np.float32).astype(bf),
            "b1": np.asarray(b1, np.float32).reshape(128, 1),
            "b2": np.asarray(b2, np.float32).reshape(128, 1),
            "b3r": np.asarray(b3, np.float32).reshape(1, 384),
            "bop": np.asarray(bop, np.float32).reshape(3, 128).T.copy(),
        })
    return n_slots, in_maps


_CACHE = {}


def _get_graph(cfg, n_slots):
    key = (cfg.N, cfg.C, tuple(int(x) for x in n_slots))
    if key not in _CACHE:
        _CACHE[key] = build_graph(cfg, n_slots)
    return _CACHE[key]


def kernel(h, vec, coord, edge_index, edge_attr,
           Wvp, W1, b1, W2, b2, W3, b3, Wop, bop):
    cfg = FULL
    n_slots, in_maps = host_prep(cfg, h, vec, coord, edge_index, edge_attr,
                                 Wvp, W1, b1, W2, b2, W3, b3, Wop, bop)
    nc = _get_graph(cfg, n_slots)
    trace = bool(int(os.environ.get("BASS_KERNEL_TRACE", "0")))
    res = run_bass_kernel_spmd(nc, in_maps, list(range(cfg.C)), trace=trace)
    kernel.last_exec_time_ns = res.exec_time_ns
    kernel.last_results = res
    dh = np.concatenate([res.results[c]["dh"] for c in range(cfg.C)], axis=0)
    dvec = np.concatenate([res.results[c]["dvec"] for c in range(cfg.C)],
                          axis=0).reshape(cfg.N, 3, 128)
    return dh, dvec


# revision 20
# speedup vs baseline: 1.4074x; 1.0547x over previous
"""Trainium2 Bass kernel for the PaiNN-style GNN message-passing layer.

Strategy (8 NeuronCores, SPMD, no collectives):
- Node rows are block-sharded: core c owns rows [c*NLOC, (c+1)*NLOC).
- Each edge is assigned to the core owning its destination (row) node, so
  all scatter-adds are core-local.
- Per-core compacted endpoint tables (unique cols < 32768) make gather
  indices fit the int16 DMA-gather index format.
- Edge MLP runs in bf16, features-on-partitions; the final MLP layer is
  computed "flipped" (activations as the stationary operand) so messages
  come out edge-major, ready for aggregation without transposes.
- Aggregation is an exact f32 one-hot matmul into a block-persistent PSUM
  tile (one 128-row node block at a time), fused with the output epilogue.
"""

import os
import numpy as np
import ml_dtypes

from concourse import bacc, bass, tile, mybir
from concourse.bass_utils import run_bass_kernel_spmd

BF16 = mybir.dt.bfloat16
F32 = mybir.dt.float32
I16 = mybir.dt.int16
I32 = mybir.dt.int32

HID = 128
NRBF = 32
EDGE = 16
CUTOFF = 5.0
EPS = 1e-8


class Cfg:
    def __init__(self, n_nodes, n_cores, u_max, sub=512):
        self.N = n_nodes
        self.C = n_cores
        self.NLOC = n_nodes // n_cores
        assert self.NLOC * n_cores == n_nodes
        self.NBLK = (self.NLOC + 127) // 128
        self.NLOC_PAD = self.NBLK * 128
        self.U = u_max  # compacted col-table rows (multiple of 128)
        self.SUB = sub  # edge sub-chunk (<=512, PSUM bank limit)


FULL = Cfg(50000, 8, 32768)


def _silu(nc, wrk, use_silu, out, xp, sub, bias, SUB):
    if use_silu:
        nc.scalar.activation(out[:, :sub], xp[:, :sub],
                             mybir.ActivationFunctionType.Silu, bias=bias[:])
    else:  # CoreSim lacks Silu: z*sigmoid(z)
        z = wrk.tile([128, SUB], BF16, tag="slz")
        nc.scalar.activation(z[:, :sub], xp[:, :sub],
                             mybir.ActivationFunctionType.Identity, bias=bias[:])
        sg = wrk.tile([128, SUB], BF16, tag="slg")
        nc.scalar.activation(sg[:, :sub], z[:, :sub],
                             mybir.ActivationFunctionType.Sigmoid)
        nc.vector.tensor_tensor(out[:, :sub], z[:, :sub], sg[:, :sub],
                                mybir.AluOpType.mult)


def _wrap_idx(idx):
    """int16 gather-index layout: [i%16, i//16], replicated 8x to 128 rows."""
    n = len(idx)
    assert n % 16 == 0
    w = idx.astype(np.int16).reshape(n // 16, 16).T
    return np.tile(w, (8, 1)).copy()


def build_graph(cfg, n_slots, use_silu=True, b_mode="full", lean=False):
    """Build the SPMD Bass graph. n_slots: per-block edge-slot counts
    (multiples of 128, identical across cores)."""
    nc = bacc.Bacc(None, target_bir_lowering=False, debug=False)
    NLOC_PAD, U, NBLK, SUB = cfg.NLOC_PAD, cfg.U, cfg.NBLK, cfg.SUB
    ES = int(sum(n_slots))  # total edge slots
    NSUB = ES // 128

    di = lambda name, shape, dt: nc.dram_tensor(name, shape, dt, kind="ExternalInput")
    # node-side uploads
    h_loc = di("h_loc", [NLOC_PAD, 128], BF16)
    h_u = di("h_u", [U, 128], BF16)
    vec_loc_T = di("vec_loc_T", [3, 128, NLOC_PAD], BF16)
    vec_T_u = di("vec_T_u", [3, 128, U], BF16)
    # edge-side uploads
    ucol_w = di("ucol_w", [128, ES // 16], I16)
    urow_w = di("urow_w", [128, ES // 16], I16)
    row_rel = di("row_rel", [128, NSUB], BF16)
    xij_t = di("xij_t", [128, NSUB, 3], BF16)
    dense_bf = di("dense_bf", [48, ES], BF16)  # rows 0:32 d_ij, 32:48 edge_attr
    # weights
    wvp = di("wvp", [128, 384], BF16)
    w1p = di("w1p", [4, 128, 128], BF16)
    w2 = di("w2", [128, 128], BF16)
    w3 = di("w3", [128, 384], BF16)
    wop = di("wop", [128, 384], BF16)
    b1 = di("b1", [128, 1], F32)
    b2 = di("b2", [128, 1], F32)
    b3r = di("b3r", [1, 384], F32)
    bop = di("bop", [128, 3], F32)

    dh_o = nc.dram_tensor("dh", [cfg.NLOC, 128], F32, kind="ExternalOutput")
    dvec_o = nc.dram_tensor("dvec", [cfg.NLOC, 384], F32, kind="ExternalOutput")

    # internal DRAM gather tables
    # tab_A row: [h | v2_0 | v3_0 | v2_1 | v3_1 | v2_2 | v3_2] (7 x 128 bf16)
    tab_A = nc.dram_tensor("tab_A", [U, 896], BF16)
    tab_R = nc.dram_tensor("tab_R", [NLOC_PAD, 512], BF16)  # [h | vec1_v0..v2]

    with tile.TileContext(nc) as tc:
        with (
            tc.tile_pool(name="resident", bufs=1) as res,
            tc.tile_pool(name="stage", bufs=2 if lean else 3) as stg,
            tc.tile_pool(name="blk", bufs=1 if lean else 2) as blk,
            tc.tile_pool(name="work", bufs=2 if lean else 3) as wrk,
            tc.tile_pool(name="psA", bufs=2, space=bass.MemorySpace.PSUM) as psA,
            tc.tile_pool(name="psW", bufs=2, space=bass.MemorySpace.PSUM) as psW,
            tc.tile_pool(name="psM", bufs=2, space=bass.MemorySpace.PSUM) as psM,
            tc.tile_pool(name="psE", bufs=2, space=bass.MemorySpace.PSUM) as psE,
        ):
            # ---- one-time setup ----
            iota_i = res.tile([128, 128], I32)
            nc.gpsimd.iota(iota_i[:], [[1, 128]], channel_multiplier=0)
            iota_colf = res.tile([128, 128], F32)
            nc.vector.tensor_copy(iota_colf[:], iota_i[:])
            iota_p = res.tile([128, 1], I32)
            nc.gpsimd.iota(iota_p[:], [[1, 1]], channel_multiplier=1)
            iota_pf = res.tile([128, 1], F32)
            nc.vector.tensor_copy(iota_pf[:], iota_p[:])
            iota_cbf = res.tile([128, 128], BF16)
            nc.vector.tensor_copy(iota_cbf[:], iota_colf[:])
            ident_bf = res.tile([128, 128], BF16)
            nc.vector.tensor_tensor(
                ident_bf[:], iota_colf[:], iota_pf[:].to_broadcast((128, 128)),
                mybir.AluOpType.is_equal)

            ones1 = res.tile([1, 128], F32)
            nc.vector.memset(ones1[:], 1.0)
            b3row = res.tile([1, 384], F32)
            nc.sync.dma_start(b3row[:], b3r[:])
            b3p = psM.tile([128, 512], F32, tag="m")
            nc.tensor.matmul(b3p[:, 0:384], ones1[:], b3row[:])
            b3_bc = res.tile([128, 384], F32)
            nc.scalar.copy(b3_bc[:], b3p[:, 0:384])

            # weights to SBUF
            wvp_s = res.tile([128, 384], BF16)
            nc.sync.dma_start(wvp_s[:], wvp[:])
            w1_s = res.tile([128, 4, 128], BF16)
            nc.sync.dma_start(w1_s[:], w1p[:].rearrange("k p f -> p k f"))
            w2_s = res.tile([128, 128], BF16)
            nc.sync.dma_start(w2_s[:], w2[:])
            w3_s = res.tile([128, 384], BF16)
            nc.sync.dma_start(w3_s[:], w3[:])
            wop_s = res.tile([128, 384], BF16)
            nc.sync.dma_start(wop_s[:], wop[:])
            b1_s = res.tile([128, 1], F32)
            nc.sync.dma_start(b1_s[:], b1[:])
            b2_s = res.tile([128, 1], F32)
            nc.sync.dma_start(b2_s[:], b2[:])
            bop_s = res.tile([128, 3], F32)
            nc.sync.dma_start(bop_s[:], bop[:])

            # resident edge metadata
            ucol_s = res.tile([128, ES // 16], I16)
            nc.sync.dma_start(ucol_s[:], ucol_w[:])
            urow_s = res.tile([128, ES // 16], I16)
            nc.sync.dma_start(urow_s[:], urow_w[:])
            rrel_s = res.tile([128, NSUB], BF16)
            nc.sync.dma_start(rrel_s[:], row_rel[:])
            xij_s = res.tile([128, NSUB, 3], BF16)
            nc.sync.dma_start(xij_s[:], xij_t[:])

            # resident node data filled by phase A
            vdot_s = res.tile([128, cfg.NBLK, 128], BF16)
            v3_s = res.tile([128, cfg.NBLK, 384], BF16)

            # ---- phase A: local vecp, vec_dot, vec3, row table ----
            nc.sync.dma_start(
                tab_R[:].rearrange("n (t f) -> n t f", t=4)[:, 0, :], h_loc[:])
            for t in range(NLOC_PAD // 128):
                vlt = stg.tile([128, 3, 128], BF16)
                nc.sync.dma_start(
                    vlt[:],
                    vec_loc_T[:, :, 128 * t:128 * (t + 1)].rearrange(
                        "v p u -> p v u"))
                v1st = stg.tile([128, 3, 128], BF16)
                acc = None
                for v in range(3):
                    p = psM.tile([128, 512], F32, tag="m")
                    nc.tensor.matmul(p[:, 0:384], vlt[:, v, :], wvp_s[:])
                    nc.vector.tensor_copy(v1st[:, v, :], p[:, 0:128])
                    tm = wrk.tile([128, 128], F32)
                    nc.vector.tensor_tensor(tm[:], p[:, 128:256], v1st[:, v, :],
                                            mybir.AluOpType.mult)
                    if v == 0:
                        acc = tm
                    elif v == 1:
                        nc.vector.tensor_tensor(acc[:], acc[:], tm[:],
                                                mybir.AluOpType.add)
                    else:
                        nc.vector.tensor_tensor(vdot_s[:, t, :], acc[:], tm[:],
                                                mybir.AluOpType.add)
                    nc.scalar.copy(v3_s[:, t, 128 * v:128 * (v + 1)],
                                   p[:, 256:384])
                nc.sync.dma_start(
                    tab_R[128 * t:128 * (t + 1), 128:512], v1st[:])

            # ---- phase A2: compacted col tables ----
            nc.sync.dma_start(
                tab_A[:].rearrange("n (t f) -> n t f", t=7)[:, 0, :], h_u[:])
            for g in range(U // 512):  # groups of 4 u-tiles
                vtg = stg.tile([128, 3, 512], BF16)
                nc.sync.dma_start(
                    vtg[:],
                    vec_T_u[:, :, 512 * g:512 * (g + 1)].rearrange(
                        "v p u -> p v u"))
                stA = stg.tile([128, 4, 768], BF16)
                for j in range(4):
                    ut = 4 * g + j
                    for v in range(3):
                        p = psM.tile([128, 512], F32, tag="m")
                        nc.tensor.matmul(
                            p[:, 0:256], vtg[:, v, 128 * j:128 * (j + 1)],
                            wvp_s[:, 128:384])
                        nc.scalar.copy(stA[:, j, 256 * v:256 * v + 128],
                                       p[:, 0:128])
                        nc.scalar.copy(stA[:, j, 256 * v + 128:256 * v + 256],
                                       p[:, 128:256])
                uslc = slice(512 * g, 512 * (g + 1))
                nc.sync.dma_start(
                    tab_A[uslc, 128:896].rearrange("(j p) f -> p j f", p=128),
                    stA[:])

            # ---- phase B: per node-block edge pipeline + fused epilogue ----
            ones_bf = res.tile([128, 128], BF16)
            nc.vector.memset(ones_bf[:], 1.0)
            zed = res.tile([128, 512], BF16)
            nc.vector.memset(zed[:], 0.0)
            if b_mode == "tables":
                for b in range(NBLK):
                    rows = min(128, cfg.NLOC - 128 * b)
                    t = wrk.tile([128, 64], F32, tag="tt")
                    nc.sync.dma_start(
                        t[:], tab_R[128 * b:128 * (b + 1), 0:128].bitcast(F32))
                    nc.sync.dma_start(dh_o[128 * b:128 * b + rows, 0:64],
                                      t[0:rows, :])
            s_off = 0  # slot offset (multiple of 128)
            for b in range(NBLK if b_mode != "tables" else 0):
                ns = int(n_slots[b])
                if ns == 0:
                    continue

                agg = psA.tile([128, 512], F32)
                nc.tensor.matmul(agg[:], ones_bf[:], zed[:], start=True,
                                 stop=False, skip_group_check=True)
                nq = ns // 128
                for c0 in range(0, ns, SUB):
                    sub = min(SUB, ns - c0)
                    ia = (s_off + c0) // 16
                    ib = (s_off + c0 + sub) // 16
                    gA_t = blk.tile([128, 7 * SUB], BF16, tag="gA")
                    gA = gA_t[:, :7 * sub].rearrange("p (t s) -> p t s", t=7)
                    nc.gpsimd.dma_gather(gA, tab_A[:],
                                         ucol_s[:, ia:ib], sub, sub, 896,
                                         transpose=True)
                    gR_t = blk.tile([128, 4 * SUB], BF16, tag="gR")
                    gR = gR_t[:, :4 * sub].rearrange("p (t s) -> p t s", t=4)
                    nc.gpsimd.dma_gather(gR, tab_R[:],
                                         urow_s[:, ia:ib], sub, sub, 512,
                                         transpose=True)
                    Ssub = wrk.tile([128, 4, 128], BF16, tag="Ssub")
                    for ql in range(sub // 128):
                        q = (s_off + c0) // 128 + ql
                        nc.vector.tensor_tensor(
                            Ssub[:, ql, :],
                            rrel_s[:, q:q + 1].to_broadcast((128, 128)),
                            iota_cbf[:], mybir.AluOpType.is_equal)
                    # cross = sum_v vec1row_v * vec2col_v  (feature-major)
                    cr = wrk.tile([128, SUB], BF16)
                    tt = wrk.tile([128, SUB], BF16)
                    nc.vector.tensor_tensor(
                        cr[:, :sub], gR[:, 1, :sub], gA[:, 1, :sub],
                        mybir.AluOpType.mult)
                    nc.vector.tensor_tensor(
                        tt[:, :sub], gR[:, 2, :sub], gA[:, 3, :sub],
                        mybir.AluOpType.mult)
                    nc.vector.tensor_tensor(cr[:, :sub], cr[:, :sub], tt[:, :sub],
                                            mybir.AluOpType.add)
                    nc.vector.tensor_tensor(
                        tt[:, :sub], gR[:, 3, :sub], gA[:, 5, :sub],
                        mybir.AluOpType.mult)
                    nc.vector.tensor_tensor(cr[:, :sub], cr[:, :sub], tt[:, :sub],
                                            mybir.AluOpType.add)
                    # k3 = [d_ij(32) | edge_attr(16)], 48-partition matmul
                    k3 = wrk.tile([48, SUB], BF16)
                    nc.sync.dma_start(k3[:, :sub],
                                      dense_bf[:, s_off + c0:s_off + c0 + sub])
                    # L1
                    x1p = psW.tile([128, SUB], F32, tag="w")
                    nc.tensor.matmul(x1p[:, :sub], w1_s[:, 0, :],
                                     gR[:, 0, :sub], start=True, stop=False)
                    nc.tensor.matmul(x1p[:, :sub], w1_s[:, 1, :],
                                     gA[:, 0, :sub], start=False, stop=False)
                    nc.tensor.matmul(x1p[:, :sub], w1_s[:, 2, :], cr[:, :sub],
                                     start=False, stop=False)
                    nc.tensor.matmul(x1p[:, :sub], w1_s[0:48, 3, :], k3[:, :sub],
                                     start=False, stop=True)
                    x1 = wrk.tile([128, SUB], BF16)
                    _silu(nc, wrk, use_silu, x1, x1p, sub, b1_s, SUB)
                    # L2
                    x2p = psW.tile([128, SUB], F32, tag="w")
                    nc.tensor.matmul(x2p[:, :sub], w2_s[:], x1[:, :sub])
                    x2 = wrk.tile([128, SUB], BF16)
                    _silu(nc, wrk, use_silu, x2, x2p, sub, b2_s, SUB)
                    # L3 flipped + aggregation per 128-edge subtile
                    for ql in range(sub // 128):
                        q = (s_off + c0) // 128 + ql
                        qb = (c0 // 128) + ql
                        mp = psM.tile([128, 512], F32, tag="m")
                        nc.tensor.matmul(mp[:, 0:384],
                                         x2[:, 128 * ql:128 * (ql + 1)], w3_s[:])
                        msg = wrk.tile([128, 384], BF16)
                        nc.vector.tensor_tensor(msg[:], mp[:, 0:384], b3_bc[:],
                                                mybir.AluOpType.add)
                        # vec3col to edge-major via TE transposes
                        v3p = psE.tile([128, 1024], BF16, tag="e")
                        for v in range(3):
                            nc.tensor.transpose(
                                v3p[:, 128 * v:128 * (v + 1)],
                                gA[:, 2 + 2 * v, 128 * ql:128 * (ql + 1)],
                                ident_bf[:])
                        v3e = wrk.tile([128, 384], BF16, tag="v3e")
                        nc.scalar.copy(v3e[:], v3p[:, 0:384])
                        # vec_msg = vec3col*m_v + x_ij*m_x  (edge-major)
                        t1 = wrk.tile([128, 3, 128], BF16)
                        nc.vector.tensor_tensor(
                            t1[:],
                            v3e[:].rearrange("p (v f) -> p v f", v=3),
                            msg[:, 128:256].rearrange("p (a f) -> p a f", a=1).to_broadcast(
                                (128, 3, 128)),
                            mybir.AluOpType.mult)
                        t2 = wrk.tile([128, 3, 128], BF16)
                        nc.vector.tensor_tensor(
                            t2[:],
                            xij_s[:, q, :].rearrange("p (v a) -> p v a", a=1).to_broadcast(
                                (128, 3, 128)),
                            msg[:, 256:384].rearrange("p (a f) -> p a f", a=1).to_broadcast(
                                (128, 3, 128)),
                            mybir.AluOpType.mult)
                        vm = wrk.tile([128, 3, 128], BF16)
                        nc.vector.tensor_tensor(vm[:], t1[:], t2[:],
                                                mybir.AluOpType.add)
                        # one-hot scatter
                        S = wrk.tile([128, 128], BF16)
                        nc.vector.tensor_tensor(
                            S[:], rrel_s[:, q:q + 1].to_broadcast((128, 128)),
                            iota_colf[:], mybir.AluOpType.is_equal)
                        first = False
                        last = qb == nq - 1
                        nc.tensor.matmul(agg[:, 0:128], Ssub[:, ql, :],
                                         msg[:, 0:128],
                                         start=first, stop=last,
                                         skip_group_check=True)
                        nc.tensor.matmul(
                            agg[:, 128:512], Ssub[:, ql, :],
                            vm[:].rearrange("p v f -> p (v f)"),
                            start=first, stop=last, skip_group_check=True)

                # ---- epilogue for block b ----
                hag = wrk.tile([128, 128], BF16)
                nc.vector.tensor_copy(hag[:], agg[:, 0:128])
                hagT_p = psE.tile([128, 1024], BF16, tag="e")
                nc.tensor.transpose(hagT_p[:, 0:128], hag[:], ident_bf[:])
                hagT = wrk.tile([128, 128], BF16)
                nc.scalar.copy(hagT[:], hagT_p[:, 0:128])
                oT = []
                for j in range(3):
                    op = psE.tile([128, 512], F32, tag="e")
                    nc.tensor.matmul(op[:, 0:128], wop_s[:, 128 * j:128 * (j + 1)],
                                     hagT[:])
                    osb = wrk.tile([128, 128], BF16)
                    nc.scalar.activation(osb[:], op[:, 0:128],
                                         mybir.ActivationFunctionType.Identity,
                                         bias=bop_s[:, j:j + 1])
                    otp = psE.tile([128, 1024], BF16, tag="e")
                    nc.tensor.transpose(otp[:, 0:128], osb[:], ident_bf[:])
                    ot = wrk.tile([128, 128], BF16)
                    nc.scalar.copy(ot[:], otp[:, 0:128])
                    oT.append(ot)
                rows = min(128, cfg.NLOC - 128 * b)
                dht = wrk.tile([128, 128], F32)
                nc.vector.tensor_tensor(dht[:], vdot_s[:, b, :], oT[1][:],
                                        mybir.AluOpType.mult)
                nc.vector.tensor_tensor(dht[:], dht[:], oT[2][:],
                                        mybir.AluOpType.add)
                nc.sync.dma_start(dh_o[128 * b:128 * b + rows, :],
                                  dht[0:rows, :])
                vma = wrk.tile([128, 3, 128], F32)
                nc.vector.tensor_tensor(
                    vma[:], v3_s[:, b, :].rearrange("p (v f) -> p v f", v=3),
                    oT[0][:].rearrange("p (a f) -> p a f", a=1).to_broadcast(
                        (128, 3, 128)),
                    mybir.AluOpType.mult)
                dvt = wrk.tile([128, 3, 128], F32)
                nc.vector.tensor_tensor(
                    dvt[:], vma[:],
                    agg[:, 128:512].rearrange("p (v f) -> p v f", v=3),
                    mybir.AluOpType.add)
                nc.sync.dma_start(dvec_o[128 * b:128 * b + rows, :],
                                  dvt[0:rows, :, :].rearrange("p v f -> p (v f)"))
                s_off += ns

    nc.compile()
    return nc


def host_prep(cfg, h, vec, coord, edge_index, edge_attr,
              Wvp, W1, b1, W2, b2, W3, b3, Wop, bop):
    """Shard + lay out all inputs. Returns (n_slots, in_maps)."""
    C, NLOC, NBLK, U = cfg.C, cfg.NLOC, cfg.NBLK, cfg.U
    bf = ml_dtypes.bfloat16
    row = np.asarray(edge_index[0], np.int64)
    col = np.asarray(edge_index[1], np.int64)
    E = row.shape[0]
    h = np.asarray(h, np.float32)
    vec = np.asarray(vec, np.float32)
    coord = np.asarray(coord, np.float32)
    edge_attr = np.asarray(edge_attr, np.float32)

    x_ij = coord[row] - coord[col]
    d = np.sqrt((x_ij * x_ij).sum(-1) + EPS)
    offs = np.linspace(0.0, CUTOFF, NRBF, dtype=np.float32)
    coeff = np.float32(-0.5 / (offs[1] - offs[0]) ** 2)
    d_ij = np.exp(coeff * (d[:, None] - offs[None, :]) ** 2).astype(np.float32)

    core = row // NLOC
    lrow = (row - core * NLOC).astype(np.int64)
    blk = lrow // 128

    # per-core, per-block edge lists
    per = []
    cnt = np.zeros((C, NBLK), np.int64)
    for c in range(C):
        m = np.nonzero(core == c)[0]
        order = np.argsort(lrow[m], kind="stable")
        e = m[order]
        per.append(e)
        cb = np.bincount(blk[e], minlength=NBLK)
        cnt[c] = cb
    n_slots = (np.ceil(cnt.max(axis=0) / 128).astype(np.int64) * 128)
    n_slots = np.maximum(n_slots, 128)
    ES = int(n_slots.sum())
    NSUB = ES // 128
    starts = np.concatenate([[0], np.cumsum(n_slots)[:-1]])

    # MLP weight repack: k-tiles [h_row, h_col, cross, (d_ij|edge_attr|0)]
    W1 = np.asarray(W1, np.float32)
    w1p = np.zeros((4, 128, 128), np.float32)
    w1p[0] = W1[0:128]
    w1p[1] = W1[128:256]
    w1p[2] = W1[288:416]
    w1p[3, 0:32] = W1[256:288]
    w1p[3, 32:48] = W1[416:432]

    in_maps = []
    for c in range(C):
        e = per[c]
        lr = lrow[e]
        bl = blk[e]
        # slot positions
        pos = np.empty(len(e), np.int64)
        off = np.zeros(NBLK, np.int64)
        # edges are sorted by lrow hence by block; place sequentially per block
        for b in range(NBLK):
            k = np.nonzero(bl == b)[0]
            pos[k] = starts[b] + np.arange(len(k))
        ucols, uinv = np.unique(col[e], return_inverse=True)
        nu = len(ucols)
        assert nu <= U, f"core {c}: {nu} unique cols > {U}"

        ucol_a = np.zeros(ES, np.int64)
        urow_a = np.zeros(ES, np.int64)
        rrel_a = np.full(ES, -1.0, np.float32)
        xij_a = np.zeros((ES, 3), np.float32)
        dense_a = np.zeros((48, ES), np.float32)
        ucol_a[pos] = uinv
        urow_a[pos] = lr
        rrel_a[pos] = (lr - 128 * bl).astype(np.float32)
        xij_a[pos] = x_ij[e]
        dense_a[0:32, pos] = d_ij[e].T
        dense_a[32:48, pos] = edge_attr[e].T

        nlp = cfg.NLOC_PAD
        h_loc = np.zeros((nlp, 128), np.float32)
        h_loc[0:NLOC] = h[c * NLOC:(c + 1) * NLOC]
        vl = np.zeros((3, 128, nlp), np.float32)
        vl[:, :, 0:NLOC] = vec[c * NLOC:(c + 1) * NLOC].transpose(1, 2, 0)
        h_uu = np.zeros((U, 128), np.float32)
        h_uu[0:nu] = h[ucols]
        vtu = np.zeros((3, 128, U), np.float32)
        vtu[:, :, 0:nu] = vec[ucols].transpose(1, 2, 0)

        in_maps.append({
            "h_loc": h_loc.astype(bf),
            "h_u": h_uu.astype(bf),
            "vec_loc_T": vl.astype(bf),
            "vec_T_u": vtu.astype(bf),
            "ucol_w": _wrap_idx(ucol_a),
            "urow_w": _wrap_idx(urow_a),
            "row_rel": rrel_a.reshape(NSUB, 128).T.astype(bf),
            "xij_t": xij_a.reshape(NSUB, 128, 3).transpose(1, 0, 2).astype(bf),
            "dense_bf": dense_a.astype(bf),
            "wvp": np.asarray(Wvp, np.float32).astype(bf),
            "w1p": w1p.astype(bf),
            "w2": np.asarray(W2, np.float32).astype(bf),
            "w3": np.asarray(W3, np.float32).astype(bf),
            "wop": np.asarray(Wop, np.float32).astype(bf),
            "b1": np.asarray(b1, np.float32).reshape(128, 1),
            "b2": np.asarray(b2, np.float32).reshape(128, 1),
            "b3r": np.asarray(b3, np.float32).reshape(1, 384),
            "bop": np.asarray(bop, np.float32).reshape(3, 128).T.copy(),
        })
    return n_slots, in_maps


_CACHE = {}


def _get_graph(cfg, n_slots):
    key = (cfg.N, cfg.C, tuple(int(x) for x in n_slots))
    if key not in _CACHE:
        _CACHE[key] = build_graph(cfg, n_slots)
    return _CACHE[key]


def kernel(h, vec, coord, edge_index, edge_attr,
           Wvp, W1, b1, W2, b2, W3, b3, Wop, bop):
    cfg = FULL
    n_slots, in_maps = host_prep(cfg, h, vec, coord, edge_index, edge_attr,
                                 Wvp, W1, b1, W2, b2, W3, b3, Wop, bop)
    nc = _get_graph(cfg, n_slots)
    trace = bool(int(os.environ.get("BASS_KERNEL_TRACE", "0")))
    res = run_bass_kernel_spmd(nc, in_maps, list(range(cfg.C)), trace=trace)
    kernel.last_exec_time_ns = res.exec_time_ns
    kernel.last_results = res
    dh = np.concatenate([res.results[c]["dh"] for c in range(cfg.C)], axis=0)
    dvec = np.concatenate([res.results[c]["dvec"] for c in range(cfg.C)],
                          axis=0).reshape(cfg.N, 3, 128)
    return dh, dvec
